# revision 6
# baseline (speedup 1.0000x reference)
"""Trainium2 Bass kernel for nn_ATTHScore (hyperbolic attention KNN scoring).

Self-contained: shards the full inputs across 8 NeuronCores (2 chunks of
1024 rows per core - pure data parallel), runs a Bass/Tile kernel per core,
gathers the full (16, 1024, 1024) score tensor.

Math notes (algebraically identical to the reference, numerically closer to
f64 truth):
  With t = tanh(sqrt(c)*||v||), xv = <x, v/||v||>, A = c*||x||^2, c2 = 1-A:
    den = 1 - 2*sqrt(c)*t*xv + A*t^2
    u   = clip(c2*(1-t^2)/den, umin, 1)        # = 1 - artanh_arg^2
    artanh(arg) = ln(1+sqrt(1-u)) - ln(u)/2
    score = -(4/c)*artanh^2 + head_bias + tail_bias
  using the gyro-identity  num^2 = denom*(c1-c2)  which collapses the
  reference's cancellation-prone num/denom expression.

ACT table-set discipline (switches cost ~2.7us): 1/sqrt(x) is computed as
Exp(-0.5*Ln(x)) so phase A lives in natural_log_exp_and_others; Sigmoid and
all Tanh live in sigmoid_and_others; only the B-phase big Sqrt(1-u) uses
sqrt_and_others.
"""

import numpy as np

import concourse.bacc as bacc
import concourse.mybir as mybir
import concourse.tile as tile
import concourse.dve_ops as dve_ops
from concourse import masks
from concourse.dve_spec import Spec, Src0, Src1, C0, C1, C2, One, sq, maxx, minn, lower
from concourse.dve_uop import DveOpSpec
from concourse.bass_utils import run_bass_kernel_spmd
from contextlib import ExitStack

dt = mybir.dt
AF = mybir.ActivationFunctionType
ALU = mybir.AluOpType

# ---------------------------------------------------------------- constants
NCORES = 8
D = 512            # feature dim
CS = 1024          # chunk_size
NS = 1024          # neg_sample_size
NCHUNK = 16        # total chunks
CPC = NCHUNK // NCORES   # chunks per core = 2
BC = CPC * CS      # rows per core = 2048
NT = BC // 128     # row tiles per core = 16
TPC = CS // 128    # row tiles per chunk = 8
F32 = dt.float32

BALL = float(np.float32(1.0 - 1e-5))
UMIN = float(np.float32(1.0 - np.float64(np.float32(1.0 - 1e-5)) ** 2))
MINN = 1e-15

# ------------------------------------------------------- custom DVE ops


def _register_op(name: str, spec: Spec) -> "dve_ops.DveOp":
    for existing in dve_ops.OPS:
        if existing.name == name:
            return existing
    shas = {}
    for ver in ("v3", "v4"):
        uops = lower(spec, ver=ver)
        shas[ver] = DveOpSpec(name=name, opcode=0, uops=uops, rd1_en=True).sha(ver)
    op = dve_ops.DveOp(name, spec, subdim=False, uops_sha=shas)
    dve_ops.OPS.append(op)
    dve_ops.CUSTOM_DVE_SPECS[name] = spec
    dve_ops._SUB_OPCODE_FOR_NAME[name] = max(dve_ops._SUB_OPCODE_FOR_NAME.values()) + 1
    assert dve_ops._SUB_OPCODE_FOR_NAME[name] < 0x20
    return op


# den = 1 + (mm*t)*C0 + t^2*C1     (C0 = -2*sqrt(c), C1 = A)
HYP_DEN = _register_op("HYP_DEN", Spec(
    body=(Src0 * Src1) * C0 + sq(Src1) * C1 + One,
    reference=lambda in0, in1, s0, s1, imm2: in0 * in1 * s0 + in1 * in1 * s1 + 1.0,
))
# u = min(max((1 - t^2)*C0*rden, C2), 1)     (C0 = c2, C2 = umin)
HYP_U = _register_op("HYP_U", Spec(
    body=minn(maxx(((One - sq(Src1)) * C0) * Src0, C2), One),
    reference=lambda in0, in1, s0, s1, imm2: np.minimum(
        np.maximum((1.0 - in1 * in1) * s0 * in0, imm2), 1.0),
))
# score-tb = (l1 - l2*C2)^2*C0 + C1          (C0 = -4/c, C1 = hb, C2 = 0.5)
HYP_SCORE = _register_op("HYP_SCORE", Spec(
    body=sq(Src0 - Src1 * C2) * C0 + C1,
    reference=lambda in0, in1, s0, s1, imm2: (in0 - in1 * imm2) ** 2 * s0 + s1,
))
# out = Src0*C0 + Src1*C1  (per-partition weighted sum)
HYP_WSUM = _register_op("HYP_WSUM", Spec(
    body=Src0 * C0 + Src1 * C1,
    reference=lambda in0, in1, s0, s1, imm2: in0 * s0 + in1 * s1,
))
# out = sq(Src0) + sq(Src1)  (pair norm^2)
HYP_N2 = _register_op("HYP_N2", Spec(
    body=sq(Src0) + sq(Src1),
    reference=lambda in0, in1, s0, s1, imm2: in0 * in0 + in1 * in1,
))


# ---------------------------------------------------------------- kernel IR


def build_nc(debug: bool = False):
    nc = bacc.Bacc("TRN2", target_bir_lowering=False, debug=False,
                   num_devices=NCORES)
    P = 128

    head_in = nc.declare_dram_parameter("head", [BC, D], F32, isOutput=False)
    hb_in = nc.declare_dram_parameter("head_bias", [BC, 1], F32, isOutput=False)
    rel_in = nc.declare_dram_parameter("rel", [BC, D], F32, isOutput=False)
    rd_in = nc.declare_dram_parameter("rel_diag", [BC, 2 * D], F32, isOutput=False)
    curv_in = nc.declare_dram_parameter("curvature", [BC, 1], F32, isOutput=False)
    ctx_in = nc.declare_dram_parameter("context", [BC, D], F32, isOutput=False)
    scale_in = nc.declare_dram_parameter("scale", [1, 1], F32, isOutput=False)
    tail_in = nc.declare_dram_parameter("tail", [BC, D], F32, isOutput=False)
    tb_in = nc.declare_dram_parameter("tail_bias", [BC, 1], F32, isOutput=False)
    score_out = nc.declare_dram_parameter("score", [BC, NS], F32, isOutput=True)
    vn_scr = nc.dram_tensor("vn_scratch", [BC, 1], F32)
    if debug:
        res_out = nc.declare_dram_parameter("dbg_res", [BC, D], F32, isOutput=True)
        sh_out = nc.declare_dram_parameter("dbg_sheets", [128, 6 * NT], F32, isOutput=True)

    def pairv(ap):
        return ap.rearrange("p (k two) -> p k two", two=2)

    with tile.TileContext(nc) as tc, ExitStack() as ctx:
        cpool = ctx.enter_context(tc.tile_pool(name="const", bufs=1))
        spool = ctx.enter_context(tc.tile_pool(name="sheets", bufs=1))
        apool = ctx.enter_context(tc.tile_pool(name="aflow", bufs=2))
        hold = ctx.enter_context(tc.tile_pool(name="hold", bufs=TPC))
        bpool = ctx.enter_context(tc.tile_pool(name="bflow", bufs=2))
        bchain = ctx.enter_context(tc.tile_pool(name="bchain", bufs=2))
        xpool = ctx.enter_context(tc.tile_pool(name="xmats", bufs=1))
        pp_tp = ctx.enter_context(tc.tile_pool(name="ps_tp", bufs=2, space="PSUM"))
        pp_mm = ctx.enter_context(tc.tile_pool(name="ps_mm", bufs=2, space="PSUM"))

        ident = cpool.tile([P, P], F32)
        masks.make_identity(nc, ident[:])

        # ---- sheets: per-row scalars, col t = row-tile t, partition = row%128
        def sheet(nm, cols=NT):
            return spool.tile([P, cols], F32, tag=nm, name=nm)

        names = ("kcurv khb srot sref xy0 c_s sqc_s rsqc_s rc_s "
                 "m2sqc_s m4c_s A_s c2_s alpha_s beta_s wrot_s wref_s "
                 "scr_s1 scr_s2 scr_s3 scr_s4 scr_s5 scr_s6 vn2_sh vn_sh").split()
        (kcurv, khb, srot, sref, xy0, c_s, sqc_s, rsqc_s, rc_s,
         m2sqc_s, m4c_s, A_s, c2_s, alpha_s, beta_s, wrot_s, wref_s,
         scr_s1, scr_s2, scr_s3, scr_s4, scr_s5, scr_s6, vn2_sh, vn_sh) = [
            sheet(nm) for nm in names]
        # paired sheets (128, 2*NT): [att-side | rel-side]
        norms2 = sheet("norms2", 2 * NT)   # [na2 | nr2]
        lnn2 = sheet("lnn2", 2 * NT)
        rnorm2 = sheet("rnorm2", 2 * NT)   # 1/norm
        zpair = sheet("zpair", 2 * NT)     # z = sqc*norm
        tpair = sheet("tpair", 2 * NT)     # tanh(z)
        fpair = sheet("fpair", 2 * NT)     # tanh(z)/z
        spair = sheet("spair", 2 * NT)     # tanh(z)/sqc
        qpair = sheet("qpair", 2 * NT)     # (tanh(z)/sqc)^2 = x2|y2

        def pair2(sh, cc):
            # (128, 2, TPC) strided view of a (128, 2*NT) paired sheet
            return sh[:].rearrange("p (two t) -> p two t", two=2)[
                :, :, TPC * cc:TPC * cc + TPC]

        nc.sync.dma_start(kcurv[:], curv_in[:].rearrange("(t p) o -> p (t o)", p=P))
        nc.sync.dma_start(khb[:], hb_in[:].rearrange("(t p) o -> p (t o)", p=P))
        scale_bc = cpool.tile([P, 1], F32)
        nc.sync.dma_start(scale_bc[:], scale_in[:].broadcast_to((P, 1)))

        # curvature chain (all 16 cols at once; natural_log_exp set)
        nc.scalar.activation(scr_s1[:], kcurv[:], AF.Exp)
        nc.scalar.activation(c_s[:], scr_s1[:], AF.Ln, bias=1.0)
        nc.scalar.activation(scr_s1[:], c_s[:], AF.Ln)
        nc.scalar.activation(sqc_s[:], scr_s1[:], AF.Exp, scale=0.5)
        nc.scalar.activation(rsqc_s[:], scr_s1[:], AF.Exp, scale=-0.5)
        nc.vector.tensor_tensor(rc_s[:], rsqc_s[:], rsqc_s[:], ALU.mult)
        nc.vector.tensor_scalar(m2sqc_s[:], sqc_s[:], -2.0, None, ALU.mult)
        nc.vector.tensor_scalar(m4c_s[:], rc_s[:], -4.0, None, ALU.mult)

        rel_tiles = {}
        rot_tiles = {}
        ref_tiles = {}
        att_tiles = {}

        def a_sweep(t):
            """Load row-tile, build rot/ref (held), dots, rel-norm."""
            r0 = t * 128
            ht = apool.tile([P, D], F32, tag="h")
            ct = apool.tile([P, D], F32, tag="ctx")
            rdt = apool.tile([P, 2 * D], F32, tag="rd")
            rlt = hold.tile([P, D], F32, tag="rel")
            rel_tiles[t] = rlt
            nc.sync.dma_start(ht[:], head_in[r0:r0 + 128, :])
            nc.sync.dma_start(ct[:], ctx_in[r0:r0 + 128, :])
            nc.sync.dma_start(rdt[:], rd_in[r0:r0 + 128, :])
            nc.sync.dma_start(rlt[:], rel_in[r0:r0 + 128, :])

            # pair-normalize rel_diag in place: gn = rd / sqrt(a^2+b^2)
            n2 = apool.tile([P, D], F32, tag="nA")
            nc.vector._custom_dve(HYP_N2, out=n2[:], in0=rdt[:, 0:2 * D:2],
                                  in1=rdt[:, 1:2 * D:2])
            lnv = apool.tile([P, D], F32, tag="nB")
            nc.scalar.activation(lnv[:], n2[:], AF.Ln)
            rsq = apool.tile([P, D], F32, tag="nA")
            nc.scalar.activation(rsq[:], lnv[:], AF.Exp, scale=-0.5)
            nc.vector.tensor_tensor(
                pairv(rdt[:]), pairv(rdt[:]),
                rsq[:].unsqueeze(-1).broadcast_to((P, D, 2)), ALU.mult)

            # products (rot half on DVE, ref half on GPSIMD)
            hsw = pairv(ht[:])[:, :, ::-1]
            pr = hold.tile([P, D], F32, tag="rot")
            qr = apool.tile([P, D], F32, tag="qr")
            pf = hold.tile([P, D], F32, tag="ref")
            qf = apool.tile([P, D], F32, tag="qf")
            rot_tiles[t] = pr
            ref_tiles[t] = pf
            nc.vector.tensor_tensor(pr[:], rdt[:, 0:D], ht[:], ALU.mult)
            nc.vector.tensor_tensor(pairv(qr[:]), pairv(rdt[:, 0:D]), hsw, ALU.mult)
            nc.gpsimd.tensor_tensor(pf[:], rdt[:, D:2 * D], ht[:], ALU.mult)
            nc.gpsimd.tensor_tensor(pairv(qf[:]), pairv(rdt[:, D:2 * D]), hsw, ALU.mult)

            # combines (in place): pr becomes rot, pf becomes ref
            nc.vector.tensor_tensor(pr[:, 0:D:2], pr[:, 0:D:2], pr[:, 1:D:2], ALU.subtract)
            nc.vector.tensor_tensor(pr[:, 1:D:2], qr[:, 0:D:2], qr[:, 1:D:2], ALU.add)
            nc.gpsimd.tensor_tensor(pf[:, 0:D:2], pf[:, 0:D:2], pf[:, 1:D:2], ALU.add)
            nc.gpsimd.tensor_tensor(pf[:, 1:D:2], qf[:, 1:D:2], qf[:, 0:D:2], ALU.subtract)

            # attention dots + rel norm^2 (fused multiply-reduce on DVE)
            scr = apool.tile([P, D], F32, tag="scr")
            nc.vector.affine_mul_reduce(scr[:], srot[:, t:t + 1], ct[:], pr[:], 1.0, 0.0)
            scr2 = apool.tile([P, D], F32, tag="scr")
            nc.vector.affine_mul_reduce(scr2[:], sref[:, t:t + 1], ct[:], pf[:], 1.0, 0.0)
            scr3 = apool.tile([P, D], F32, tag="scr")
            nc.vector.affine_mul_reduce(scr3[:], norms2[:, NT + t:NT + t + 1],
                                        rlt[:], rlt[:], 1.0, 0.0)

        def s1_sheets(cc):
            sl = slice(TPC * cc, TPC * cc + TPC)
            nc.vector.tensor_tensor(scr_s1[:, sl], srot[:, sl], sref[:, sl], ALU.subtract)
            nc.scalar.activation(wrot_s[:, sl], scr_s1[:, sl], AF.Sigmoid,
                                 scale=scale_bc[:])
            nc.vector.tensor_scalar(wref_s[:, sl], wrot_s[:, sl], -1.0, 1.0,
                                    ALU.mult, ALU.add)

        def a2_sweep(t):
            att = hold.tile([P, D], F32, tag="att")
            att_tiles[t] = att
            nc.vector._custom_dve(HYP_WSUM, out=att[:], in0=rot_tiles[t][:],
                                  in1=ref_tiles[t][:],
                                  s0=wrot_s[:, t:t + 1], s1=wref_s[:, t:t + 1])
            scr = apool.tile([P, D], F32, tag="scr")
            nc.vector.affine_mul_reduce(scr[:], norms2[:, t:t + 1], att[:], att[:],
                                        1.0, 0.0)
            scr2 = apool.tile([P, D], F32, tag="scr")
            nc.vector.affine_mul_reduce(scr2[:], xy0[:, t:t + 1], att[:],
                                        rel_tiles[t][:], 1.0, 0.0)
            del rot_tiles[t], ref_tiles[t]

        def s2_sheets(cc):
            sl = slice(TPC * cc, TPC * cc + TPC)
            TT = nc.vector.tensor_tensor
            TS = nc.vector.tensor_scalar
            STT = nc.vector.scalar_tensor_tensor

            # paired norm chain: norm = exp(.5 ln n2), 1/norm = exp(-.5 ln n2)
            nc.scalar.activation(pair2(lnn2, cc), pair2(norms2, cc), AF.Ln)
            nc.scalar.activation(pair2(rnorm2, cc), pair2(lnn2, cc), AF.Exp, scale=-0.5)
            nc.scalar.activation(pair2(zpair, cc), pair2(lnn2, cc), AF.Exp, scale=0.5)
            sqcb = sqc_s[:, sl].unsqueeze(1).broadcast_to((P, 2, TPC))
            rsqcb = rsqc_s[:, sl].unsqueeze(1).broadcast_to((P, 2, TPC))
            TT(pair2(zpair, cc), pair2(zpair, cc), sqcb, ALU.mult)
            nc.scalar.activation(pair2(tpair, cc), pair2(zpair, cc), AF.Tanh)
            # f = tanh(z)/z = tanh(z) * (1/norm) * (1/sqc)
            TT(pair2(fpair, cc), pair2(tpair, cc), pair2(rnorm2, cc), ALU.mult)
            TT(pair2(fpair, cc), pair2(fpair, cc), rsqcb, ALU.mult)
            # s = tanh(z)/sqc ; q = s^2  (x2 | y2)
            TT(pair2(spair, cc), pair2(tpair, cc), rsqcb, ALU.mult)
            TT(pair2(qpair, cc), pair2(spair, cc), pair2(spair, cc), ALU.mult)

            q2 = qpair[:].rearrange("p (two t) -> p two t", two=2)
            x2 = q2[:, 0, TPC * cc:TPC * cc + TPC]
            y2 = q2[:, 1, TPC * cc:TPC * cc + TPC]
            f2 = fpair[:].rearrange("p (two t) -> p two t", two=2)
            fa = f2[:, 0, TPC * cc:TPC * cc + TPC]
            fr = f2[:, 1, TPC * cc:TPC * cc + TPC]

            xy = scr_s1
            TT(xy[:, sl], fa, fr, ALU.mult)
            TT(xy[:, sl], xy[:, sl], xy0[:, sl], ALU.mult)

            cxy2, cy2, cx2 = scr_s2, scr_s3, scr_s4
            STT(cxy2[:, sl], xy[:, sl], 2.0, c_s[:, sl], ALU.mult, ALU.mult)
            TT(cy2[:, sl], c_s[:, sl], y2, ALU.mult)
            TT(cx2[:, sl], c_s[:, sl], x2, ALU.mult)
            ccx2y2 = scr_s5
            TT(ccx2y2[:, sl], cx2[:, sl], cy2[:, sl], ALU.mult)
            n1, dm = scr_s6, scr_s3  # cy2 consumed after n1
            TT(n1[:, sl], cxy2[:, sl], cy2[:, sl], ALU.add)
            TS(n1[:, sl], n1[:, sl], 1.0, None, ALU.add)
            TT(dm[:, sl], cxy2[:, sl], ccx2y2[:, sl], ALU.add)
            TS(dm[:, sl], dm[:, sl], 1.0, MINN, ALU.add, ALU.max)
            n2c = scr_s2  # cxy2 dead
            TS(n2c[:, sl], cx2[:, sl], -1.0, 1.0, ALU.mult, ALU.add)
            rdm = scr_s5  # ccx2y2 dead
            nc.vector.reciprocal_approx_accurate(rdm[:, sl], dm[:, sl], scr_s4[:, sl])
            a0, b0 = scr_s3, scr_s4  # dm, cx2 dead
            TT(a0[:, sl], n1[:, sl], rdm[:, sl], ALU.mult)
            TT(b0[:, sl], n2c[:, sl], rdm[:, sl], ALU.mult)

            # rn2 = a0^2*x2 + b0^2*y2 + 2*a0*b0*xy
            p1, p2 = scr_s6, scr_s2  # n1, n2c dead
            TT(p1[:, sl], a0[:, sl], a0[:, sl], ALU.mult)
            TT(p1[:, sl], p1[:, sl], x2, ALU.mult)
            TT(p2[:, sl], b0[:, sl], b0[:, sl], ALU.mult)
            TT(p2[:, sl], p2[:, sl], y2, ALU.mult)
            rn2 = scr_s5  # rdm dead
            TT(rn2[:, sl], p1[:, sl], p2[:, sl], ALU.add)
            ab = scr_s6  # p1 dead
            TT(ab[:, sl], a0[:, sl], b0[:, sl], ALU.mult)
            TT(ab[:, sl], ab[:, sl], xy[:, sl], ALU.mult)
            STT(rn2[:, sl], ab[:, sl], 2.0, rn2[:, sl], ALU.mult, ALU.add)
            # g = min(maxn/rn, 1) with 1/rn = exp(-0.5 ln rn2)
            lnr = scr_s2
            nc.scalar.activation(lnr[:, sl], rn2[:, sl], AF.Ln)
            rrn = scr_s6
            nc.scalar.activation(rrn[:, sl], lnr[:, sl], AF.Exp, scale=-0.5)
            g = scr_s2
            STT(g[:, sl], rrn[:, sl], BALL, rsqc_s[:, sl], ALU.mult, ALU.mult)
            TS(g[:, sl], g[:, sl], 1.0, None, ALU.min)

            TT(alpha_s[:, sl], g[:, sl], a0[:, sl], ALU.mult)
            TT(alpha_s[:, sl], alpha_s[:, sl], fa, ALU.mult)
            TT(beta_s[:, sl], g[:, sl], b0[:, sl], ALU.mult)
            TT(beta_s[:, sl], beta_s[:, sl], fr, ALU.mult)
            gg = scr_s6  # rrn dead
            TT(gg[:, sl], g[:, sl], g[:, sl], ALU.mult)
            TT(gg[:, sl], gg[:, sl], rn2[:, sl], ALU.mult)
            TT(A_s[:, sl], gg[:, sl], c_s[:, sl], ALU.mult)
            TS(c2_s[:, sl], A_s[:, sl], -1.0, 1.0, ALU.mult, ALU.add)

        def a3_sweep(t, cc, xT):
            q = t - TPC * cc
            res = apool.tile([P, D], F32, tag="res")
            nc.vector._custom_dve(HYP_WSUM, out=res[:], in0=att_tiles[t][:],
                                  in1=rel_tiles[t][:],
                                  s0=alpha_s[:, t:t + 1], s1=beta_s[:, t:t + 1])
            if debug:
                nc.sync.dma_start(res_out[t * 128:(t + 1) * 128, :], res[:])
            ptp = pp_tp.tile([P, D], F32, tag="tp")
            for dk in range(4):
                nc.tensor.transpose(ptp[:, dk * 128:(dk + 1) * 128],
                                    res[:, dk * 128:(dk + 1) * 128], ident[:])
            nc.vector.tensor_copy(
                xT[:].rearrange("p (dk n) -> p dk n", dk=4)[:, :, q * 128:(q + 1) * 128],
                ptp[:].rearrange("p (dk n) -> p dk n", dk=4))
            del att_tiles[t], rel_tiles[t]

        def b_chunk(cc, xT):
            # ---- prep: tail norms + raw transposes (streamed)
            vhatT = xpool.tile([P, 4 * NS], F32, tag="vhatT")
            for q in range(TPC):
                r0 = cc * CS + q * 128
                vt = apool.tile([P, D], F32, tag="vtl")
                nc.sync.dma_start(vt[:], tail_in[r0:r0 + 128, :])
                scr = apool.tile([P, D], F32, tag="scr")
                nc.vector.affine_mul_reduce(
                    scr[:], vn2_sh[:, TPC * cc + q:TPC * cc + q + 1],
                    vt[:], vt[:], 1.0, 0.0)
                ptp = pp_tp.tile([P, D], F32, tag="tp")
                for dk in range(4):
                    nc.tensor.transpose(ptp[:, dk * 128:(dk + 1) * 128],
                                        vt[:, dk * 128:(dk + 1) * 128], ident[:])
                nc.vector.tensor_copy(
                    vhatT[:].rearrange("p (dk n) -> p dk n", dk=4)[:, :, q * 128:(q + 1) * 128],
                    ptp[:].rearrange("p (dk n) -> p dk n", dk=4))
            sl = slice(TPC * cc, TPC * cc + TPC)
            # vn = exp(0.5 ln vn2)  (stay in lnexp set)
            nc.scalar.activation(scr_s1[:, sl], vn2_sh[:, sl], AF.Ln)
            nc.scalar.activation(vn_sh[:, sl], scr_s1[:, sl], AF.Exp, scale=0.5)
            # vn sheet slice -> DRAM scratch -> partition-broadcast load
            scr_rows = vn_scr[cc * CS:(cc + 1) * CS, :]
            nc.sync.dma_start(scr_rows.rearrange("(t p) o -> p (t o)", p=P),
                              vn_sh[:, sl])
            vn_b = bpool.tile([P, NS], F32, tag="vn_b")
            nc.sync.dma_start(
                vn_b[:],
                scr_rows.rearrange("(o n) one -> o (n one)", o=1
                                   ).broadcast_to((P, NS)))
            rv_b = bpool.tile([P, NS], F32, tag="rv_b")
            nc.vector.reciprocal_approx_fast(rv_b[:], vn_b[:])
            # normalize vhatT in place (one big strided TT)
            nc.vector.tensor_tensor(
                vhatT[:].rearrange("p (dk n) -> p dk n", dk=4),
                vhatT[:].rearrange("p (dk n) -> p dk n", dk=4),
                rv_b[:].unsqueeze(1).broadcast_to((P, 4, NS)), ALU.mult)

            tb_b = bpool.tile([P, NS], F32, tag="tb_b")
            nc.sync.dma_start(
                tb_b[:],
                tb_in[cc * CS:(cc + 1) * CS, :].rearrange("(o n) one -> o (n one)", o=1
                                                          ).broadcast_to((P, NS)))

            # ---- main sweep
            for q in range(TPC):
                t = TPC * cc + q
                tcol = slice(t, t + 1)
                pmm = pp_mm.tile([P, NS], F32, tag="mm")
                for ns in range(2):
                    for dk in range(4):
                        nc.tensor.matmul(
                            pmm[:, ns * 512:(ns + 1) * 512],
                            xT[:, dk * 1024 + q * 128: dk * 1024 + (q + 1) * 128],
                            vhatT[:, dk * 1024 + ns * 512: dk * 1024 + (ns + 1) * 512],
                            start=(dk == 0), stop=(dk == 3))
                tt = bchain.tile([P, NS], F32, tag="bt")
                nc.scalar.activation(tt[:], vn_b[:], AF.Tanh, scale=sqc_s[:, tcol])
                den = bchain.tile([P, NS], F32, tag="bA")
                nc.vector._custom_dve(HYP_DEN, out=den[:], in0=pmm[:], in1=tt[:],
                                      s0=m2sqc_s[:, tcol], s1=A_s[:, tcol])
                rden = bchain.tile([P, NS], F32, tag="bB")
                nc.vector.reciprocal_approx_fast(rden[:], den[:])
                u = bchain.tile([P, NS], F32, tag="bC")
                nc.vector._custom_dve(HYP_U, out=u[:], in0=rden[:], in1=tt[:],
                                      s0=c2_s[:, tcol], imm2=UMIN)
                s_ = bchain.tile([P, NS], F32, tag="bA")
                nc.scalar.activation(s_[:], u[:], AF.Sqrt, bias=1.0, scale=-1.0)
                l1 = bchain.tile([P, NS], F32, tag="bB")
                nc.scalar.activation(l1[:], s_[:], AF.Ln, bias=1.0)
                l2 = bchain.tile([P, NS], F32, tag="bC")
                nc.scalar.activation(l2[:], u[:], AF.Ln)
                sc0 = bchain.tile([P, NS], F32, tag="bA")
                nc.vector._custom_dve(HYP_SCORE, out=sc0[:], in0=l1[:], in1=l2[:],
                                      s0=m4c_s[:, tcol], s1=khb[:, tcol], imm2=0.5)
                outt = bchain.tile([P, NS], F32, tag="bB")
                nc.gpsimd.tensor_tensor(outt[:], sc0[:], tb_b[:], ALU.add)
                nc.sync.dma_start(score_out[t * 128:(t + 1) * 128, :], outt[:])

        # ---------------- emission: chunk-major for pipelining
        for cc in range(CPC):
            for t in range(TPC * cc, TPC * cc + TPC):
                a_sweep(t)
            s1_sheets(cc)
            for t in range(TPC * cc, TPC * cc + TPC):
                a2_sweep(t)
            s2_sheets(cc)
            xT = xpool.tile([P, 4 * NS], F32, tag="xT")
            for t in range(TPC * cc, TPC * cc + TPC):
                a3_sweep(t, cc, xT)
            b_chunk(cc, xT)

        if debug:
            dbg = [c_s, sqc_s, A_s, c2_s, alpha_s, beta_s]
            for i, sh in enumerate(dbg):
                nc.sync.dma_start(sh_out[:, i * NT:(i + 1) * NT], sh[:])

    nc.finalize()
    return nc


_NC_CACHE = {}


def _get_nc(debug=False):
    if debug not in _NC_CACHE:
        _NC_CACHE[debug] = build_nc(debug)
    return _NC_CACHE[debug]


def kernel(head, head_bias, rel, rel_diag, curvature, context, scale, tail,
           tail_bias, chunk_size, neg_sample_size, _debug=False, _trace=False):
    cs = int(chunk_size)
    ns = int(neg_sample_size)
    assert cs == CS and ns == NS, (cs, ns)
    head = np.ascontiguousarray(np.asarray(head, np.float32))
    head_bias = np.ascontiguousarray(np.asarray(head_bias, np.float32))
    rel = np.ascontiguousarray(np.asarray(rel, np.float32))
    rel_diag = np.ascontiguousarray(np.asarray(rel_diag, np.float32))
    curvature = np.ascontiguousarray(np.asarray(curvature, np.float32))
    context = np.ascontiguousarray(np.asarray(context, np.float32))
    scale = np.ascontiguousarray(np.asarray(scale, np.float32)).reshape(1, 1)
    tail = np.ascontiguousarray(np.asarray(tail, np.float32))
    tail_bias = np.ascontiguousarray(np.asarray(tail_bias, np.float32))

    nc = _get_nc(_debug)
    in_maps = []
    for core in range(NCORES):
        r = slice(core * BC, (core + 1) * BC)
        in_maps.append({
            "head": head[r], "head_bias": head_bias[r], "rel": rel[r],
            "rel_diag": rel_diag[r], "curvature": curvature[r],
            "context": context[r], "scale": scale, "tail": tail[r],
            "tail_bias": tail_bias[r],
        })
    res = run_bass_kernel_spmd(nc, in_maps, core_ids=list(range(NCORES)),
                               trace=_trace)
    score = np.concatenate([res.results[c]["score"] for c in range(NCORES)], axis=0)
    out = score.reshape(NCHUNK, CS, NS)
    if _debug:
        dbg_res = np.concatenate([res.results[c]["dbg_res"] for c in range(NCORES)], 0)
        dbg_sheets = [res.results[c]["dbg_sheets"] for c in range(NCORES)]
        return out, dbg_res, dbg_sheets
    if _trace:
        return out, res
    return out


# revision 45
# speedup vs baseline: 278.3355x; 278.3355x over previous
"""Trainium2 Bass kernel for nn_ATTHScore (hyperbolic attention KNN scoring).

Self-contained: shards the full inputs across 8 NeuronCores (2 chunks of
1024 rows per core - pure data parallel), runs a Bass/Tile kernel per core,
gathers the full (16, 1024, 1024) score tensor.

Math notes (algebraically identical to the reference, numerically closer to
f64 truth):
  With t = tanh(sqrt(c)*||v||), xv = <x, v/||v||>, A = c*||x||^2, c2 = 1-A:
    den = 1 - 2*sqrt(c)*t*xv + A*t^2
    u   = clip(c2*(1-t^2)/den, umin, 1)        # = 1 - artanh_arg^2
    artanh(arg) = ln(1+sqrt(1-u)) - ln(u)/2
    score = -(4/c)*artanh^2 + head_bias + tail_bias
  using the gyro-identity  num^2 = denom*(c1-c2)  which collapses the
  reference's cancellation-prone num/denom expression.

ACT table-set discipline (switches cost ~2.7us): 1/sqrt(x) is computed as
Exp(-0.5*Ln(x)) so phase A lives in natural_log_exp_and_others; Sigmoid and
all Tanh live in sigmoid_and_others; only the B-phase big Sqrt(1-u) uses
sqrt_and_others.
"""

import numpy as np

import concourse.bacc as bacc
import concourse.mybir as mybir
import concourse.tile as tile
import concourse.dve_ops as dve_ops
from concourse import masks
from concourse.dve_spec import Spec, Src0, Src1, C0, C1, C2, One, sq, maxx, minn, lower
from concourse.dve_uop import DveOpSpec
from concourse.bass_utils import run_bass_kernel_spmd
from contextlib import ExitStack

dt = mybir.dt
AF = mybir.ActivationFunctionType
ALU = mybir.AluOpType

# ---------------------------------------------------------------- constants
NCORES = 8
D = 512            # feature dim
CS = 1024          # chunk_size
NS = 1024          # neg_sample_size
NCHUNK = 16        # total chunks
CPC = NCHUNK // NCORES   # chunks per core = 2
BC = CPC * CS      # rows per core = 2048
NT = BC // 128     # row tiles per core = 16
TPC = CS // 128    # row tiles per chunk = 8
F32 = dt.float32

BALL = float(np.float32(1.0 - 1e-5))
UMIN = float(np.float32(1.0 - np.float64(np.float32(1.0 - 1e-5)) ** 2))
MINN = 1e-15

# ------------------------------------------------------- custom DVE ops


def _register_op(name: str, spec: Spec) -> "dve_ops.DveOp":
    for existing in dve_ops.OPS:
        if existing.name == name:
            return existing
    shas = {}
    for ver in ("v3", "v4"):
        uops = lower(spec, ver=ver)
        shas[ver] = DveOpSpec(name=name, opcode=0, uops=uops, rd1_en=True).sha(ver)
    op = dve_ops.DveOp(name, spec, subdim=False, uops_sha=shas)
    dve_ops.OPS.append(op)
    dve_ops.CUSTOM_DVE_SPECS[name] = spec
    dve_ops._SUB_OPCODE_FOR_NAME[name] = max(dve_ops._SUB_OPCODE_FOR_NAME.values()) + 1
    assert dve_ops._SUB_OPCODE_FOR_NAME[name] < 0x20
    return op


# den = 1 + (mm*t)*C0 + t^2*C1     (C0 = -2*sqrt(c), C1 = A)
HYP_DEN = _register_op("HYP_DEN", Spec(
    body=(Src0 * Src1) * C0 + sq(Src1) * C1 + One,
    reference=lambda in0, in1, s0, s1, imm2: in0 * in1 * s0 + in1 * in1 * s1 + 1.0,
))
# u = min(max((1 - t^2)*C0*rden, C2), 1)     (C0 = c2, C2 = umin)
HYP_U = _register_op("HYP_U", Spec(
    body=minn(maxx(((One - sq(Src1)) * C0) * Src0, C2), One),
    reference=lambda in0, in1, s0, s1, imm2: np.minimum(
        np.maximum((1.0 - in1 * in1) * s0 * in0, imm2), 1.0),
))
# score-tb = (l1 - l2*C2)^2*C0 + C1          (C0 = -4/c, C1 = hb, C2 = 0.5)
HYP_SCORE = _register_op("HYP_SCORE", Spec(
    body=sq(Src0 - Src1 * C2) * C0 + C1,
    reference=lambda in0, in1, s0, s1, imm2: (in0 - in1 * imm2) ** 2 * s0 + s1,
))
# out = Src0*C0 + Src1*C1  (per-partition weighted sum)
HYP_WSUM = _register_op("HYP_WSUM", Spec(
    body=Src0 * C0 + Src1 * C1,
    reference=lambda in0, in1, s0, s1, imm2: in0 * s0 + in1 * s1,
))
# out = sq(Src0) + sq(Src1)  (pair norm^2)
HYP_N2 = _register_op("HYP_N2", Spec(
    body=sq(Src0) + sq(Src1),
    reference=lambda in0, in1, s0, s1, imm2: in0 * in0 + in1 * in1,
))
# tanh from exp: t = (E - 1) * rE1   (E pre-clamped; rE1 = 1/(E+1))
HYP_TFE = _register_op("HYP_TFE", Spec(
    body=(Src0 - One) * Src1,
    reference=lambda in0, in1, s0, s1, imm2: (in0 - 1.0) * in1,
))


# ---------------------------------------------------------------- kernel IR


def build_nc(debug: bool = False):
    nc = bacc.Bacc("TRN2", target_bir_lowering=False, debug=False,
                   num_devices=NCORES)
    P = 128

    head_in = nc.declare_dram_parameter("head", [BC, D], F32, isOutput=False)
    hb_in = nc.declare_dram_parameter("head_bias", [BC, 1], F32, isOutput=False)
    rel_in = nc.declare_dram_parameter("rel", [BC, D], F32, isOutput=False)
    rd_in = nc.declare_dram_parameter("rel_diag", [BC, 2 * D], F32, isOutput=False)
    curv_in = nc.declare_dram_parameter("curvature", [BC, 1], F32, isOutput=False)
    ctx_in = nc.declare_dram_parameter("context", [BC, D], F32, isOutput=False)
    scale_in = nc.declare_dram_parameter("scale", [1, 1], F32, isOutput=False)
    tail_in = nc.declare_dram_parameter("tail", [BC, D], F32, isOutput=False)
    tb_in = nc.declare_dram_parameter("tail_bias", [BC, 1], F32, isOutput=False)
    score_out = nc.declare_dram_parameter("score", [BC, NS], F32, isOutput=True)
    vn_scr = nc.dram_tensor("vn_scratch", [BC, 1], F32)
    if debug:
        res_out = nc.declare_dram_parameter("dbg_res", [BC, D], F32, isOutput=True)
        sh_out = nc.declare_dram_parameter("dbg_sheets", [128, 6 * NT], F32, isOutput=True)

    def pairv(ap):
        return ap.rearrange("p (k two) -> p k two", two=2)

    with tile.TileContext(nc) as tc, ExitStack() as ctx:
        cpool = ctx.enter_context(tc.tile_pool(name="const", bufs=1))
        spool = ctx.enter_context(tc.tile_pool(name="sheets", bufs=1))
        apool = ctx.enter_context(tc.tile_pool(name="aflow", bufs=2))
        inpool = ctx.enter_context(tc.tile_pool(name="influx", bufs=2))
        hold = ctx.enter_context(tc.tile_pool(name="hold", bufs=TPC))
        wpool = ctx.enter_context(tc.tile_pool(name="wcols", bufs=3))
        bpool = ctx.enter_context(tc.tile_pool(name="bflow", bufs=2))
        bpool1 = ctx.enter_context(tc.tile_pool(name="bflow1", bufs=1))
        bchain = ctx.enter_context(tc.tile_pool(name="bchain", bufs=3))
        bheld = ctx.enter_context(tc.tile_pool(name="bheld", bufs=5))
        scrpool = ctx.enter_context(tc.tile_pool(name="scrp", bufs=1))
        xpool = ctx.enter_context(tc.tile_pool(name="xmats", bufs=1))
        xtpool = ctx.enter_context(tc.tile_pool(name="xtmat", bufs=2))
        pp_tp = ctx.enter_context(tc.tile_pool(name="ps_tp", bufs=2, space="PSUM"))
        pp_mm = ctx.enter_context(tc.tile_pool(name="ps_mm", bufs=3, space="PSUM"))

        ident = cpool.tile([P, P], F32)
        masks.make_identity(nc, ident[:])

        # ---- sheets: per-row scalars, col t = row-tile t, partition = row%128
        def sheet(nm, cols=NT):
            return spool.tile([P, cols], F32, tag=nm, name=nm)

        names = ("kcurv khb srot sref xy0 c_s sqc_s rsqc_s rc_s "
                 "m2sqc_s m4c_s A_s c2_s alpha_s beta_s wrot_s wref_s "
                 "scr_s1 scr_s2 scr_s3 scr_s4 scr_s5 scr_s6 vn2_sh vn_sh").split()
        (kcurv, khb, srot, sref, xy0, c_s, sqc_s, rsqc_s, rc_s,
         m2sqc_s, m4c_s, A_s, c2_s, alpha_s, beta_s, wrot_s, wref_s,
         scr_s1, scr_s2, scr_s3, scr_s4, scr_s5, scr_s6, vn2_sh, vn_sh) = [
            sheet(nm) for nm in names]
        # paired sheets (128, 2*NT): [att-side | rel-side]
        norms2 = sheet("norms2", 2 * NT)   # [na2 | nr2]
        lnn2 = sheet("lnn2", 2 * NT)
        rnorm2 = sheet("rnorm2", 2 * NT)   # 1/norm
        zpair = sheet("zpair", 2 * NT)     # z = sqc*norm
        tpair = sheet("tpair", 2 * NT)     # tanh(z)
        fpair = sheet("fpair", 2 * NT)     # tanh(z)/z
        spair = sheet("spair", 2 * NT)     # tanh(z)/sqc
        qpair = sheet("qpair", 2 * NT)     # (tanh(z)/sqc)^2 = x2|y2

        def pair2(sh, cc):
            # (128, 2, TPC) strided view of a (128, 2*NT) paired sheet
            return sh[:].rearrange("p (two t) -> p two t", two=2)[
                :, :, TPC * cc:TPC * cc + TPC]

        nc.sync.dma_start(kcurv[:], curv_in[:].rearrange("(t p) o -> p (t o)", p=P))
        nc.sync.dma_start(khb[:], hb_in[:].rearrange("(t p) o -> p (t o)", p=P))
        scale_bc = cpool.tile([P, 1], F32)
        nc.sync.dma_start(scale_bc[:], scale_in[:].broadcast_to((P, 1)))
        nscale_bc = cpool.tile([P, 1], F32)
        nc.vector.tensor_scalar(nscale_bc[:], scale_bc[:], -1.0, None, ALU.mult)

        ACT_LOAD(LNEXP_SET)
        # curvature chain (all 16 cols at once; natural_log_exp set)
        nc.scalar.activation(scr_s1[:], kcurv[:], AF.Exp)
        nc.scalar.activation(c_s[:], scr_s1[:], AF.Ln, bias=1.0)
        nc.scalar.activation(scr_s1[:], c_s[:], AF.Ln)
        nc.scalar.activation(sqc_s[:], scr_s1[:], AF.Exp, scale=0.5)
        nc.scalar.activation(rsqc_s[:], scr_s1[:], AF.Exp, scale=-0.5)
        nc.vector.tensor_tensor(rc_s[:], rsqc_s[:], rsqc_s[:], ALU.mult)
        nc.vector.tensor_scalar(m2sqc_s[:], sqc_s[:], -2.0, None, ALU.mult)
        nc.vector.tensor_scalar(m4c_s[:], rc_s[:], -4.0, None, ALU.mult)

        att_tiles = {}

        def a_sweep(t):
            """Load row-tile, build rot/ref (held), dots, rel-norm."""
            r0 = t * 128
            ht = inpool.tile([P, D], F32, tag="h")
            ct = inpool.tile([P, D], F32, tag="ctx")
            rdt = inpool.tile([P, 2 * D], F32, tag="rd")

            nc.sync.dma_start(ht[:], head_in[r0:r0 + 128, :])
            nc.sync.dma_start(ct[:], ctx_in[r0:r0 + 128, :])
            nc.sync.dma_start(rdt[:], rd_in[r0:r0 + 128, :])

            # pair-normalize rel_diag in place: gn = rd / sqrt(a^2+b^2)
            n2 = apool.tile([P, D], F32, tag="nA")
            nc.vector._custom_dve(HYP_N2, out=n2[:], in0=rdt[:, 0:2 * D:2],
                                  in1=rdt[:, 1:2 * D:2])
            lnv = apool.tile([P, D], F32, tag="nB")
            nc.scalar.activation(lnv[:], n2[:], AF.Ln)
            rsq = apool.tile([P, D], F32, tag="nA")
            nc.scalar.activation(rsq[:], lnv[:], AF.Exp, scale=-0.5)
            nc.gpsimd.tensor_tensor(
                pairv(rdt[:]), pairv(rdt[:]),
                rsq[:].unsqueeze(-1).broadcast_to((P, D, 2)), ALU.mult)

            # products (rot half on DVE, ref half on GPSIMD)
            hsw = pairv(ht[:])[:, :, ::-1]
            pr = apool.tile([P, D], F32, tag="rot")
            qr = apool.tile([P, D], F32, tag="qr")
            pf = apool.tile([P, D], F32, tag="ref")
            qf = apool.tile([P, D], F32, tag="qf")
            nc.vector.tensor_tensor(pr[:], rdt[:, 0:D], ht[:], ALU.mult)
            nc.vector.tensor_tensor(pairv(qr[:]), pairv(rdt[:, 0:D]), hsw, ALU.mult)
            nc.gpsimd.tensor_tensor(pf[:], rdt[:, D:2 * D], ht[:], ALU.mult)
            nc.gpsimd.tensor_tensor(pairv(qf[:]), pairv(rdt[:, D:2 * D]), hsw, ALU.mult)

            # combines (in place): pr becomes rot, pf becomes ref
            nc.vector.tensor_tensor(pr[:, 0:D:2], pr[:, 0:D:2], pr[:, 1:D:2], ALU.subtract)
            nc.vector.tensor_tensor(pr[:, 1:D:2], qr[:, 0:D:2], qr[:, 1:D:2], ALU.add)
            nc.gpsimd.tensor_tensor(pf[:, 0:D:2], pf[:, 0:D:2], pf[:, 1:D:2], ALU.add)
            nc.gpsimd.tensor_tensor(pf[:, 1:D:2], qf[:, 1:D:2], qf[:, 0:D:2], ALU.subtract)

            # attention dots (fused multiply-reduce on DVE)
            scr = scrpool.tile([P, D], F32, tag="scr")
            nc.vector.affine_mul_reduce(scr[:], srot[:, t:t + 1], ct[:], pr[:], 1.0, 0.0)
            scr2 = scrpool.tile([P, D], F32, tag="scr")
            nc.vector.affine_mul_reduce(scr2[:], sref[:, t:t + 1], ct[:], pf[:], 1.0, 0.0)

            # per-tile attention weights (exp-form sigmoid, lnexp set)
            wd = wpool.tile([P, 1], F32, tag="wd")
            nc.vector.tensor_tensor(wd[:], srot[:, t:t + 1], sref[:, t:t + 1],
                                    ALU.subtract)
            we = wpool.tile([P, 1], F32, tag="we")
            ACT(we[:], wd[:], AF.Exp, scale=nscale_bc[:])
            nc.vector.tensor_scalar(we[:], we[:], 1.0, None, ALU.add)
            wrot = wpool.tile([P, 1], F32, tag="wrot")
            nc.vector.reciprocal_approx_fast(wrot[:], we[:])
            wref = wpool.tile([P, 1], F32, tag="wref")
            nc.vector.tensor_scalar(wref[:], wrot[:], -1.0, 1.0, ALU.mult, ALU.add)

            # att + reductions (na2, xy0, nr2)
            rlt = inpool.tile([P, D], F32, tag="rel")
            nc.sync.dma_start(rlt[:], rel_in[r0:r0 + 128, :])
            att = hold.tile([P, D], F32, tag="att")
            att_tiles[t] = att
            nc.vector._custom_dve(HYP_WSUM, out=att[:], in0=pr[:], in1=pf[:],
                                  s0=wrot[:], s1=wref[:])
            scr4 = scrpool.tile([P, D], F32, tag="scrA")
            ACT(scr4[:], att[:], AF.Square, accum_out=norms2[:, t:t + 1])
            scr5 = scrpool.tile([P, D], F32, tag="scr")
            nc.vector.affine_mul_reduce(scr5[:], xy0[:, t:t + 1], att[:], rlt[:],
                                        1.0, 0.0)
            scr6 = scrpool.tile([P, D], F32, tag="scrA")
            ACT(scr6[:], rlt[:], AF.Square, accum_out=norms2[:, NT + t:NT + t + 1])


        def s2_sheets(cc):
            sl = slice(TPC * cc, TPC * cc + TPC)
            TT = nc.vector.tensor_tensor
            TS = nc.vector.tensor_scalar
            STT = nc.vector.scalar_tensor_tensor

            # paired norm chain: norm = exp(.5 ln n2), 1/norm = exp(-.5 ln n2)
            nc.scalar.activation(pair2(lnn2, cc), pair2(norms2, cc), AF.Ln)
            nc.scalar.activation(pair2(rnorm2, cc), pair2(lnn2, cc), AF.Exp, scale=-0.5)
            nc.scalar.activation(pair2(zpair, cc), pair2(lnn2, cc), AF.Exp, scale=0.5)
            sqcb = sqc_s[:, sl].unsqueeze(1).broadcast_to((P, 2, TPC))
            rsqcb = rsqc_s[:, sl].unsqueeze(1).broadcast_to((P, 2, TPC))
            TT(pair2(zpair, cc), pair2(zpair, cc), sqcb, ALU.mult)
            # tanh via exp (stay in lnexp set): E = exp(2z); t = (E-1)/(E+1)
            nc.scalar.activation(pair2(lnn2, cc), pair2(zpair, cc), AF.Exp, scale=2.0)
            TS(pair2(lnn2, cc), pair2(lnn2, cc), 3.0e37, None, ALU.min)
            TS(pair2(zpair, cc), pair2(lnn2, cc), 1.0, None, ALU.add)
            nc.vector.reciprocal_approx_fast(pair2(norms2, cc),
                                                 pair2(zpair, cc))
            nc.vector._custom_dve(HYP_TFE, out=pair2(tpair, cc),
                                  in0=pair2(lnn2, cc), in1=pair2(norms2, cc))
            # f = tanh(z)/z = tanh(z) * (1/norm) * (1/sqc)
            TT(pair2(fpair, cc), pair2(tpair, cc), pair2(rnorm2, cc), ALU.mult)
            TT(pair2(fpair, cc), pair2(fpair, cc), rsqcb, ALU.mult)
            # s = tanh(z)/sqc ; q = s^2  (x2 | y2)
            TT(pair2(spair, cc), pair2(tpair, cc), rsqcb, ALU.mult)
            TT(pair2(qpair, cc), pair2(spair, cc), pair2(spair, cc), ALU.mult)

            q2 = qpair[:].rearrange("p (two t) -> p two t", two=2)
            x2 = q2[:, 0, TPC * cc:TPC * cc + TPC]
            y2 = q2[:, 1, TPC * cc:TPC * cc + TPC]
            f2 = fpair[:].rearrange("p (two t) -> p two t", two=2)
            fa = f2[:, 0, TPC * cc:TPC * cc + TPC]
            fr = f2[:, 1, TPC * cc:TPC * cc + TPC]

            xy = scr_s1
            TT(xy[:, sl], fa, fr, ALU.mult)
            TT(xy[:, sl], xy[:, sl], xy0[:, sl], ALU.mult)

            cxy2, cy2, cx2 = scr_s2, scr_s3, scr_s4
            STT(cxy2[:, sl], xy[:, sl], 2.0, c_s[:, sl], ALU.mult, ALU.mult)
            TT(cy2[:, sl], c_s[:, sl], y2, ALU.mult)
            TT(cx2[:, sl], c_s[:, sl], x2, ALU.mult)
            ccx2y2 = scr_s5
            TT(ccx2y2[:, sl], cx2[:, sl], cy2[:, sl], ALU.mult)
            n1, dm = scr_s6, scr_s3  # cy2 consumed after n1
            TT(n1[:, sl], cxy2[:, sl], cy2[:, sl], ALU.add)
            TS(n1[:, sl], n1[:, sl], 1.0, None, ALU.add)
            TT(dm[:, sl], cxy2[:, sl], ccx2y2[:, sl], ALU.add)
            TS(dm[:, sl], dm[:, sl], 1.0, MINN, ALU.add, ALU.max)
            n2c = scr_s2  # cxy2 dead
            TS(n2c[:, sl], cx2[:, sl], -1.0, 1.0, ALU.mult, ALU.add)
            rdm = scr_s5  # ccx2y2 dead
            nc.vector.reciprocal_approx_fast(rdm[:, sl], dm[:, sl])
            a0, b0 = scr_s3, scr_s4  # dm, cx2 dead
            TT(a0[:, sl], n1[:, sl], rdm[:, sl], ALU.mult)
            TT(b0[:, sl], n2c[:, sl], rdm[:, sl], ALU.mult)

            # rn2 = a0^2*x2 + b0^2*y2 + 2*a0*b0*xy
            p1, p2 = scr_s6, scr_s2  # n1, n2c dead
            TT(p1[:, sl], a0[:, sl], a0[:, sl], ALU.mult)
            TT(p1[:, sl], p1[:, sl], x2, ALU.mult)
            TT(p2[:, sl], b0[:, sl], b0[:, sl], ALU.mult)
            TT(p2[:, sl], p2[:, sl], y2, ALU.mult)
            rn2 = scr_s5  # rdm dead
            TT(rn2[:, sl], p1[:, sl], p2[:, sl], ALU.add)
            ab = scr_s6  # p1 dead
            TT(ab[:, sl], a0[:, sl], b0[:, sl], ALU.mult)
            TT(ab[:, sl], ab[:, sl], xy[:, sl], ALU.mult)
            STT(rn2[:, sl], ab[:, sl], 2.0, rn2[:, sl], ALU.mult, ALU.add)
            # g = min(maxn/rn, 1) with 1/rn = exp(-0.5 ln rn2)
            lnr = scr_s2
            nc.scalar.activation(lnr[:, sl], rn2[:, sl], AF.Ln)
            rrn = scr_s6
            nc.scalar.activation(rrn[:, sl], lnr[:, sl], AF.Exp, scale=-0.5)
            g = scr_s2
            STT(g[:, sl], rrn[:, sl], BALL, rsqc_s[:, sl], ALU.mult, ALU.mult)
            TS(g[:, sl], g[:, sl], 1.0, None, ALU.min)

            TT(alpha_s[:, sl], g[:, sl], a0[:, sl], ALU.mult)
            TT(alpha_s[:, sl], alpha_s[:, sl], fa, ALU.mult)
            TT(beta_s[:, sl], g[:, sl], b0[:, sl], ALU.mult)
            TT(beta_s[:, sl], beta_s[:, sl], fr, ALU.mult)
            gg = scr_s6  # rrn dead
            TT(gg[:, sl], g[:, sl], g[:, sl], ALU.mult)
            TT(gg[:, sl], gg[:, sl], rn2[:, sl], ALU.mult)
            TT(A_s[:, sl], gg[:, sl], c_s[:, sl], ALU.mult)
            TS(c2_s[:, sl], A_s[:, sl], -1.0, 1.0, ALU.mult, ALU.add)

        def a3_sweep(t, cc, xT):
            q = t - TPC * cc
            rlt3 = inpool.tile([P, D], F32, tag="rel")
            nc.sync.dma_start(rlt3[:], rel_in[t * 128:(t + 1) * 128, :])
            res = apool.tile([P, D], F32, tag="res")
            nc.vector._custom_dve(HYP_WSUM, out=res[:], in0=att_tiles[t][:],
                                  in1=rlt3[:],
                                  s0=alpha_s[:, t:t + 1], s1=beta_s[:, t:t + 1])
            if debug:
                nc.sync.dma_start(res_out[t * 128:(t + 1) * 128, :], res[:])
            ptp = pp_tp.tile([P, D], F32, tag="tp")
            for dk in range(4):
                nc.tensor.transpose(ptp[:, dk * 128:(dk + 1) * 128],
                                    res[:, dk * 128:(dk + 1) * 128], ident[:])
            nc.scalar.copy(
                xT[:].rearrange("p (dk n) -> p dk n", dk=4)[:, :, q * 128:(q + 1) * 128],
                ptp[:].rearrange("p (dk n) -> p dk n", dk=4))
            del att_tiles[t]

        def b_prep(cc):
            # ---- prep: tail norms + raw transposes (streamed)
            vhatT = xpool.tile([P, 4 * NS], F32, tag="vhatT")
            for q in range(TPC):
                r0 = cc * CS + q * 128
                vt = inpool.tile([P, D], F32, tag="vtl")
                nc.sync.dma_start(vt[:], tail_in[r0:r0 + 128, :])
                scr = scrpool.tile([P, D], F32, tag="scrA")
                ACT(scr[:], vt[:], AF.Square,
                    accum_out=vn2_sh[:, TPC * cc + q:TPC * cc + q + 1])
                ptp = pp_tp.tile([P, D], F32, tag="tp")
                for dk in range(4):
                    nc.tensor.transpose(ptp[:, dk * 128:(dk + 1) * 128],
                                        vt[:, dk * 128:(dk + 1) * 128], ident[:])
                nc.scalar.copy(
                    vhatT[:].rearrange("p (dk n) -> p dk n", dk=4)[:, :, q * 128:(q + 1) * 128],
                    ptp[:].rearrange("p (dk n) -> p dk n", dk=4))
            sl = slice(TPC * cc, TPC * cc + TPC)
            # vn = exp(0.5 ln vn2)  (stay in lnexp set)
            nc.scalar.activation(scr_s1[:, sl], vn2_sh[:, sl], AF.Ln)
            nc.scalar.activation(vn_sh[:, sl], scr_s1[:, sl], AF.Exp, scale=0.5)
            # vn sheet slice -> DRAM scratch -> partition-broadcast load
            scr_rows = vn_scr[cc * CS:(cc + 1) * CS, :]
            nc.sync.dma_start(scr_rows.rearrange("(t p) o -> p (t o)", p=P),
                              vn_sh[:, sl])
            vn_b = bpool.tile([P, NS], F32, tag="vn_b")
            nc.sync.dma_start(
                vn_b[:],
                scr_rows.rearrange("(o n) one -> o (n one)", o=1
                                   ).broadcast_to((P, NS)))
            rv_b = bpool1.tile([P, NS], F32, tag="rv_b")
            nc.vector.reciprocal_approx_fast(rv_b[:], vn_b[:])
            # normalize vhatT in place (one big strided TT)
            nc.vector.tensor_tensor(
                vhatT[:].rearrange("p (dk n) -> p dk n", dk=4),
                vhatT[:].rearrange("p (dk n) -> p dk n", dk=4),
                rv_b[:].unsqueeze(1).broadcast_to((P, 4, NS)), ALU.mult)

            tb_b = bpool1.tile([P, NS], F32, tag="tb_b")
            nc.sync.dma_start(
                tb_b[:],
                tb_in[cc * CS:(cc + 1) * CS, :].rearrange("(o n) one -> o (n one)", o=1
                                                          ).broadcast_to((P, NS)))
            return vhatT, vn_b, tb_b

        def b_main(cc, xT, vhatT, vn_b, tb_b, interleave=None):
            # ---- main sweep: function-major groups to batch ACT table sets
            NB = 4
            for g in range(TPC // NB):
                qs = list(range(g * NB, (g + 1) * NB))
                u_tiles = {}
                s_tiles = {}
                # pass 1: matmul, tanh, den/rden/u (ACT: Tanh only)
                for q in qs:
                    t = TPC * cc + q
                    tcol = slice(t, t + 1)
                    pmm = pp_mm.tile([P, NS], F32, tag="mm")
                    for ns in range(2):
                        for dk in range(4):
                            nc.tensor.matmul(
                                pmm[:, ns * 512:(ns + 1) * 512],
                                xT[:, dk * 1024 + q * 128: dk * 1024 + (q + 1) * 128],
                                vhatT[:, dk * 1024 + ns * 512: dk * 1024 + (ns + 1) * 512],
                                start=(dk == 0), stop=(dk == 3))
                    tt = bpool.tile([P, NS], F32, tag="bt")
                    nc.scalar.activation(tt[:], vn_b[:], AF.Tanh, scale=sqc_s[:, tcol])
                    den = bchain.tile([P, NS], F32, tag="bw1")
                    nc.vector._custom_dve(HYP_DEN, out=den[:], in0=pmm[:], in1=tt[:],
                                          s0=m2sqc_s[:, tcol], s1=A_s[:, tcol])
                    rden = bchain.tile([P, NS], F32, tag="bw2")
                    nc.vector.reciprocal_approx_fast(rden[:], den[:])
                    u = bheld.tile([P, NS], F32, tag="bu")
                    nc.vector._custom_dve(HYP_U, out=u[:], in0=rden[:], in1=tt[:],
                                          s0=c2_s[:, tcol], imm2=UMIN)
                    u_tiles[q] = u
                # pass 2: s = sqrt(1-u)  (ACT: Sqrt only)
                for q in qs:
                    s_ = bheld.tile([P, NS], dt.bfloat16, tag="bs")
                    nc.scalar.activation(s_[:], u_tiles[q][:], AF.Sqrt,
                                         bias=1.0, scale=-1.0)
                    s_tiles[q] = s_
                # pass 3: ln, ln, score, +tb, store  (ACT: Ln only)
                for q in qs:
                    t = TPC * cc + q
                    tcol = slice(t, t + 1)
                    l1 = bchain.tile([P, NS], F32, tag="bw1")
                    nc.scalar.activation(l1[:], s_tiles[q][:], AF.Ln, bias=1.0)
                    l2 = bchain.tile([P, NS], F32, tag="bw2")
                    nc.scalar.activation(l2[:], u_tiles[q][:], AF.Ln)
                    sc0 = bchain.tile([P, NS], F32, tag="bw1")
                    nc.vector._custom_dve(HYP_SCORE, out=sc0[:], in0=l1[:], in1=l2[:],
                                          s0=m4c_s[:, tcol], s1=khb[:, tcol], imm2=0.5)
                    outt = bchain.tile([P, NS], F32, tag="bw2")
                    nc.gpsimd.tensor_tensor(outt[:], sc0[:], tb_b[:], ALU.add)
                    nc.sync.dma_start(score_out[t * 128:(t + 1) * 128, :], outt[:])
                if interleave is not None:
                    interleave(g)

        # ---------------- emission: software-pipelined A phases (ACT one
        # tile ahead of DVE consumers), then B phases back to back.
        pre_state = {}
        for t in range(NT):
            a_pre(t)
            if t > 0:
                a_main(t - 1)
            if t == NT - 1:
                a_main(t)
            if t == TPC - 1:
                s2_after = True
        s2_sheets(0)
        s2_sheets(1)
        for cc in range(CPC):
            prep = b_prep(cc)
            xT = xtpool.tile([P, 4 * NS], F32, tag="xT", name="xT")
            for t in range(TPC * cc, TPC * cc + TPC):
                a3_sweep(t, cc, xT)
            b_main(cc, xT, *prep)

        if debug:
            dbg = [c_s, sqc_s, A_s, c2_s, alpha_s, beta_s]
            for i, sh in enumerate(dbg):
                nc.sync.dma_start(sh_out[:, i * NT:(i + 1) * NT], sh[:])

    nc.finalize()
    return nc


_NC_CACHE = {}


def _get_nc(debug=False):
    if debug not in _NC_CACHE:
        _NC_CACHE[debug] = build_nc(debug)
    return _NC_CACHE[debug]


def kernel(head, head_bias, rel, rel_diag, curvature, context, scale, tail,
           tail_bias, chunk_size, neg_sample_size, _debug=False, _trace=False):
    cs = int(chunk_size)
    ns = int(neg_sample_size)
    assert cs == CS and ns == NS, (cs, ns)
    head = np.ascontiguousarray(np.asarray(head, np.float32))
    head_bias = np.ascontiguousarray(np.asarray(head_bias, np.float32))
    rel = np.ascontiguousarray(np.asarray(rel, np.float32))
    rel_diag = np.ascontiguousarray(np.asarray(rel_diag, np.float32))
    curvature = np.ascontiguousarray(np.asarray(curvature, np.float32))
    context = np.ascontiguousarray(np.asarray(context, np.float32))
    scale = np.ascontiguousarray(np.asarray(scale, np.float32)).reshape(1, 1)
    tail = np.ascontiguousarray(np.asarray(tail, np.float32))
    tail_bias = np.ascontiguousarray(np.asarray(tail_bias, np.float32))

    nc = _get_nc(_debug)
    in_maps = []
    for core in range(NCORES):
        r = slice(core * BC, (core + 1) * BC)
        in_maps.append({
            "head": head[r], "head_bias": head_bias[r], "rel": rel[r],
            "rel_diag": rel_diag[r], "curvature": curvature[r],
            "context": context[r], "scale": scale, "tail": tail[r],
            "tail_bias": tail_bias[r],
        })
    res = run_bass_kernel_spmd(nc, in_maps, core_ids=list(range(NCORES)),
                               trace=_trace)
    score = np.concatenate([res.results[c]["score"] for c in range(NCORES)], axis=0)
    out = score.reshape(NCHUNK, CS, NS)
    if _debug:
        dbg_res = np.concatenate([res.results[c]["dbg_res"] for c in range(NCORES)], 0)
        dbg_sheets = [res.results[c]["dbg_sheets"] for c in range(NCORES)]
        return out, dbg_res, dbg_sheets
    if _trace:
        return out, res
    return out


# revision 47
# speedup vs baseline: 278.8532x; 1.0019x over previous
"""Trainium2 Bass kernel for nn_ATTHScore (hyperbolic attention KNN scoring).

Self-contained: shards the full inputs across 8 NeuronCores (2 chunks of
1024 rows per core - pure data parallel), runs a Bass/Tile kernel per core,
gathers the full (16, 1024, 1024) score tensor.

Math notes (algebraically identical to the reference, numerically closer to
f64 truth):
  With t = tanh(sqrt(c)*||v||), xv = <x, v/||v||>, A = c*||x||^2, c2 = 1-A:
    den = 1 - 2*sqrt(c)*t*xv + A*t^2
    u   = clip(c2*(1-t^2)/den, umin, 1)        # = 1 - artanh_arg^2
    artanh(arg) = ln(1+sqrt(1-u)) - ln(u)/2
    score = -(4/c)*artanh^2 + head_bias + tail_bias
  using the gyro-identity  num^2 = denom*(c1-c2)  which collapses the
  reference's cancellation-prone num/denom expression.

ACT table-set discipline (switches cost ~2.7us): 1/sqrt(x) is computed as
Exp(-0.5*Ln(x)) so phase A lives in natural_log_exp_and_others; Sigmoid and
all Tanh live in sigmoid_and_others; only the B-phase big Sqrt(1-u) uses
sqrt_and_others.
"""

import numpy as np

import concourse.bacc as bacc
import concourse.mybir as mybir
import concourse.tile as tile
import concourse.dve_ops as dve_ops
from concourse import masks
from concourse.dve_spec import Spec, Src0, Src1, C0, C1, C2, One, sq, maxx, minn, lower
from concourse.dve_uop import DveOpSpec
from concourse.bass_utils import run_bass_kernel_spmd
from contextlib import ExitStack

dt = mybir.dt
AF = mybir.ActivationFunctionType
ALU = mybir.AluOpType

# ---------------------------------------------------------------- constants
NCORES = 8
D = 512            # feature dim
CS = 1024          # chunk_size
NS = 1024          # neg_sample_size
NCHUNK = 16        # total chunks
CPC = NCHUNK // NCORES   # chunks per core = 2
BC = CPC * CS      # rows per core = 2048
NT = BC // 128     # row tiles per core = 16
TPC = CS // 128    # row tiles per chunk = 8
F32 = dt.float32

BALL = float(np.float32(1.0 - 1e-5))
UMIN = float(np.float32(1.0 - np.float64(np.float32(1.0 - 1e-5)) ** 2))
MINN = 1e-15

# ------------------------------------------------------- custom DVE ops


def _register_op(name: str, spec: Spec) -> "dve_ops.DveOp":
    for existing in dve_ops.OPS:
        if existing.name == name:
            return existing
    shas = {}
    for ver in ("v3", "v4"):
        uops = lower(spec, ver=ver)
        shas[ver] = DveOpSpec(name=name, opcode=0, uops=uops, rd1_en=True).sha(ver)
    op = dve_ops.DveOp(name, spec, subdim=False, uops_sha=shas)
    dve_ops.OPS.append(op)
    dve_ops.CUSTOM_DVE_SPECS[name] = spec
    dve_ops._SUB_OPCODE_FOR_NAME[name] = max(dve_ops._SUB_OPCODE_FOR_NAME.values()) + 1
    assert dve_ops._SUB_OPCODE_FOR_NAME[name] < 0x20
    return op


# den = 1 + (mm*t)*C0 + t^2*C1     (C0 = -2*sqrt(c), C1 = A)
HYP_DEN = _register_op("HYP_DEN", Spec(
    body=(Src0 * Src1) * C0 + sq(Src1) * C1 + One,
    reference=lambda in0, in1, s0, s1, imm2: in0 * in1 * s0 + in1 * in1 * s1 + 1.0,
))
# u = min(max((1 - t^2)*C0*rden, C2), 1)     (C0 = c2, C2 = umin)
HYP_U = _register_op("HYP_U", Spec(
    body=minn(maxx(((One - sq(Src1)) * C0) * Src0, C2), One),
    reference=lambda in0, in1, s0, s1, imm2: np.minimum(
        np.maximum((1.0 - in1 * in1) * s0 * in0, imm2), 1.0),
))
# score-tb = (l1 - l2*C2)^2*C0 + C1          (C0 = -4/c, C1 = hb, C2 = 0.5)
HYP_SCORE = _register_op("HYP_SCORE", Spec(
    body=sq(Src0 - Src1 * C2) * C0 + C1,
    reference=lambda in0, in1, s0, s1, imm2: (in0 - in1 * imm2) ** 2 * s0 + s1,
))
# out = Src0*C0 + Src1*C1  (per-partition weighted sum)
HYP_WSUM = _register_op("HYP_WSUM", Spec(
    body=Src0 * C0 + Src1 * C1,
    reference=lambda in0, in1, s0, s1, imm2: in0 * s0 + in1 * s1,
))
# out = sq(Src0) + sq(Src1)  (pair norm^2)
HYP_N2 = _register_op("HYP_N2", Spec(
    body=sq(Src0) + sq(Src1),
    reference=lambda in0, in1, s0, s1, imm2: in0 * in0 + in1 * in1,
))
# tanh from exp: t = (E - 1) * rE1   (E pre-clamped; rE1 = 1/(E+1))
HYP_TFE = _register_op("HYP_TFE", Spec(
    body=(Src0 - One) * Src1,
    reference=lambda in0, in1, s0, s1, imm2: (in0 - 1.0) * in1,
))
# out = max(Src0 + Src1 + 1, C2)   (n1 with C2=-inf-ish, dm with C2=MINN)
HYP_ADD1M = _register_op("HYP_ADD1M", Spec(
    body=maxx(Src0 + Src1 + One, C2),
    reference=lambda in0, in1, s0, s1, imm2: np.maximum(in0 + in1 + 1.0, imm2),
))
# out = sq(Src0) * Src1   (a0^2*x2, b0^2*y2, g^2*rn2)
HYP_SQMUL = _register_op("HYP_SQMUL", Spec(
    body=sq(Src0) * Src1,
    reference=lambda in0, in1, s0, s1, imm2: in0 * in0 * in1,
))


# ---------------------------------------------------------------- kernel IR


def build_nc(debug: bool = False):
    nc = bacc.Bacc("TRN2", target_bir_lowering=False, debug=False,
                   num_devices=NCORES)
    P = 128

    head_in = nc.declare_dram_parameter("head", [BC, D], F32, isOutput=False)
    hb_in = nc.declare_dram_parameter("head_bias", [BC, 1], F32, isOutput=False)
    rel_in = nc.declare_dram_parameter("rel", [BC, D], F32, isOutput=False)
    rd_in = nc.declare_dram_parameter("rel_diag", [BC, 2 * D], F32, isOutput=False)
    curv_in = nc.declare_dram_parameter("curvature", [BC, 1], F32, isOutput=False)
    ctx_in = nc.declare_dram_parameter("context", [BC, D], F32, isOutput=False)
    scale_in = nc.declare_dram_parameter("scale", [1, 1], F32, isOutput=False)
    tail_in = nc.declare_dram_parameter("tail", [BC, D], F32, isOutput=False)
    tb_in = nc.declare_dram_parameter("tail_bias", [BC, 1], F32, isOutput=False)
    score_out = nc.declare_dram_parameter("score", [BC, NS], F32, isOutput=True)
    vn_scr = nc.dram_tensor("vn_scratch", [BC, 1], F32)
    if debug:
        res_out = nc.declare_dram_parameter("dbg_res", [BC, D], F32, isOutput=True)
        sh_out = nc.declare_dram_parameter("dbg_sheets", [128, 6 * NT], F32, isOutput=True)

    def pairv(ap):
        return ap.rearrange("p (k two) -> p k two", two=2)

    with tile.TileContext(nc) as tc, ExitStack() as ctx:
        cpool = ctx.enter_context(tc.tile_pool(name="const", bufs=1))
        spool = ctx.enter_context(tc.tile_pool(name="sheets", bufs=1))
        apool = ctx.enter_context(tc.tile_pool(name="aflow", bufs=2))
        inpool = ctx.enter_context(tc.tile_pool(name="influx", bufs=2))
        hold = ctx.enter_context(tc.tile_pool(name="hold", bufs=TPC))
        wpool = ctx.enter_context(tc.tile_pool(name="wcols", bufs=3))
        bpool = ctx.enter_context(tc.tile_pool(name="bflow", bufs=2))
        bpool1 = ctx.enter_context(tc.tile_pool(name="bflow1", bufs=1))
        bchain = ctx.enter_context(tc.tile_pool(name="bchain", bufs=3))
        bheld = ctx.enter_context(tc.tile_pool(name="bheld", bufs=5))
        scrpool = ctx.enter_context(tc.tile_pool(name="scrp", bufs=1))
        xpool = ctx.enter_context(tc.tile_pool(name="xmats", bufs=1))
        xtpool = ctx.enter_context(tc.tile_pool(name="xtmat", bufs=2))
        pp_tp = ctx.enter_context(tc.tile_pool(name="ps_tp", bufs=2, space="PSUM"))
        pp_mm = ctx.enter_context(tc.tile_pool(name="ps_mm", bufs=3, space="PSUM"))

        ident = cpool.tile([P, P], F32)
        masks.make_identity(nc, ident[:])

        # ---- sheets: per-row scalars, col t = row-tile t, partition = row%128
        def sheet(nm, cols=NT):
            return spool.tile([P, cols], F32, tag=nm, name=nm)

        names = ("kcurv khb srot sref xy0 c_s sqc_s rsqc_s rc_s "
                 "m2sqc_s m4c_s A_s c2_s alpha_s beta_s wrot_s wref_s "
                 "scr_s1 scr_s2 scr_s3 scr_s4 scr_s5 scr_s6 vn2_sh vn_sh").split()
        (kcurv, khb, srot, sref, xy0, c_s, sqc_s, rsqc_s, rc_s,
         m2sqc_s, m4c_s, A_s, c2_s, alpha_s, beta_s, wrot_s, wref_s,
         scr_s1, scr_s2, scr_s3, scr_s4, scr_s5, scr_s6, vn2_sh, vn_sh) = [
            sheet(nm) for nm in names]
        # paired sheets (128, 2*NT): [att-side | rel-side]
        norms2 = sheet("norms2", 2 * NT)   # [na2 | nr2]
        lnn2 = sheet("lnn2", 2 * NT)
        rnorm2 = sheet("rnorm2", 2 * NT)   # 1/norm
        zpair = sheet("zpair", 2 * NT)     # z = sqc*norm
        tpair = sheet("tpair", 2 * NT)     # tanh(z)
        fpair = sheet("fpair", 2 * NT)     # tanh(z)/z
        spair = sheet("spair", 2 * NT)     # tanh(z)/sqc
        qpair = sheet("qpair", 2 * NT)     # (tanh(z)/sqc)^2 = x2|y2

        def pair2(sh, cc):
            # (128, 2, TPC) strided view of a (128, 2*NT) paired sheet
            return sh[:].rearrange("p (two t) -> p two t", two=2)[
                :, :, TPC * cc:TPC * cc + TPC]

        nc.sync.dma_start(kcurv[:], curv_in[:].rearrange("(t p) o -> p (t o)", p=P))
        nc.sync.dma_start(khb[:], hb_in[:].rearrange("(t p) o -> p (t o)", p=P))
        scale_bc = cpool.tile([P, 1], F32)
        nc.sync.dma_start(scale_bc[:], scale_in[:].broadcast_to((P, 1)))
        nscale_bc = cpool.tile([P, 1], F32)
        nc.vector.tensor_scalar(nscale_bc[:], scale_bc[:], -1.0, None, ALU.mult)

        ACT_LOAD(LNEXP_SET)
        # curvature chain (all 16 cols at once; natural_log_exp set)
        nc.scalar.activation(scr_s1[:], kcurv[:], AF.Exp)
        nc.scalar.activation(c_s[:], scr_s1[:], AF.Ln, bias=1.0)
        nc.scalar.activation(scr_s1[:], c_s[:], AF.Ln)
        nc.scalar.activation(sqc_s[:], scr_s1[:], AF.Exp, scale=0.5)
        nc.scalar.activation(rsqc_s[:], scr_s1[:], AF.Exp, scale=-0.5)
        nc.vector.tensor_tensor(rc_s[:], rsqc_s[:], rsqc_s[:], ALU.mult)
        nc.vector.tensor_scalar(m2sqc_s[:], sqc_s[:], -2.0, None, ALU.mult)
        nc.vector.tensor_scalar(m4c_s[:], rc_s[:], -4.0, None, ALU.mult)

        att_tiles = {}

        def a_sweep(t):
            """Load row-tile, build rot/ref (held), dots, rel-norm."""
            r0 = t * 128
            ht = inpool.tile([P, D], F32, tag="h")
            ct = inpool.tile([P, D], F32, tag="ctx")
            rdt = inpool.tile([P, 2 * D], F32, tag="rd")

            nc.sync.dma_start(ht[:], head_in[r0:r0 + 128, :])
            nc.sync.dma_start(ct[:], ctx_in[r0:r0 + 128, :])
            nc.sync.dma_start(rdt[:], rd_in[r0:r0 + 128, :])

            # pair-normalize rel_diag in place: gn = rd / sqrt(a^2+b^2)
            n2 = apool.tile([P, D], F32, tag="nA")
            nc.vector._custom_dve(HYP_N2, out=n2[:], in0=rdt[:, 0:2 * D:2],
                                  in1=rdt[:, 1:2 * D:2])
            lnv = apool.tile([P, D], F32, tag="nB")
            nc.scalar.activation(lnv[:], n2[:], AF.Ln)
            rsq = apool.tile([P, D], F32, tag="nA")
            nc.scalar.activation(rsq[:], lnv[:], AF.Exp, scale=-0.5)
            nc.gpsimd.tensor_tensor(
                pairv(rdt[:]), pairv(rdt[:]),
                rsq[:].unsqueeze(-1).broadcast_to((P, D, 2)), ALU.mult)

            # products (rot half on DVE, ref half on GPSIMD)
            hsw = pairv(ht[:])[:, :, ::-1]
            pr = apool.tile([P, D], F32, tag="rot")
            qr = apool.tile([P, D], F32, tag="qr")
            pf = apool.tile([P, D], F32, tag="ref")
            qf = apool.tile([P, D], F32, tag="qf")
            nc.vector.tensor_tensor(pr[:], rdt[:, 0:D], ht[:], ALU.mult)
            nc.vector.tensor_tensor(pairv(qr[:]), pairv(rdt[:, 0:D]), hsw, ALU.mult)
            nc.gpsimd.tensor_tensor(pf[:], rdt[:, D:2 * D], ht[:], ALU.mult)
            nc.gpsimd.tensor_tensor(pairv(qf[:]), pairv(rdt[:, D:2 * D]), hsw, ALU.mult)

            # combines (in place): pr becomes rot, pf becomes ref
            nc.vector.tensor_tensor(pr[:, 0:D:2], pr[:, 0:D:2], pr[:, 1:D:2], ALU.subtract)
            nc.vector.tensor_tensor(pr[:, 1:D:2], qr[:, 0:D:2], qr[:, 1:D:2], ALU.add)
            nc.gpsimd.tensor_tensor(pf[:, 0:D:2], pf[:, 0:D:2], pf[:, 1:D:2], ALU.add)
            nc.gpsimd.tensor_tensor(pf[:, 1:D:2], qf[:, 1:D:2], qf[:, 0:D:2], ALU.subtract)

            # attention dots (fused multiply-reduce on DVE)
            scr = scrpool.tile([P, D], F32, tag="scr")
            nc.vector.affine_mul_reduce(scr[:], srot[:, t:t + 1], ct[:], pr[:], 1.0, 0.0)
            scr2 = scrpool.tile([P, D], F32, tag="scr")
            nc.vector.affine_mul_reduce(scr2[:], sref[:, t:t + 1], ct[:], pf[:], 1.0, 0.0)

            # per-tile attention weights (exp-form sigmoid, lnexp set)
            wd = wpool.tile([P, 1], F32, tag="wd")
            nc.vector.tensor_tensor(wd[:], srot[:, t:t + 1], sref[:, t:t + 1],
                                    ALU.subtract)
            we = wpool.tile([P, 1], F32, tag="we")
            ACT(we[:], wd[:], AF.Exp, scale=nscale_bc[:])
            nc.vector.tensor_scalar(we[:], we[:], 1.0, None, ALU.add)
            wrot = wpool.tile([P, 1], F32, tag="wrot")
            nc.vector.reciprocal_approx_fast(wrot[:], we[:])
            wref = wpool.tile([P, 1], F32, tag="wref")
            nc.vector.tensor_scalar(wref[:], wrot[:], -1.0, 1.0, ALU.mult, ALU.add)

            # att + reductions (na2, xy0, nr2)
            rlt = inpool.tile([P, D], F32, tag="rel")
            nc.sync.dma_start(rlt[:], rel_in[r0:r0 + 128, :])
            att = hold.tile([P, D], F32, tag="att")
            att_tiles[t] = att
            nc.vector._custom_dve(HYP_WSUM, out=att[:], in0=pr[:], in1=pf[:],
                                  s0=wrot[:], s1=wref[:])
            scr4 = scrpool.tile([P, D], F32, tag="scrA")
            ACT(scr4[:], att[:], AF.Square, accum_out=norms2[:, t:t + 1])
            scr5 = scrpool.tile([P, D], F32, tag="scr")
            nc.vector.affine_mul_reduce(scr5[:], xy0[:, t:t + 1], att[:], rlt[:],
                                        1.0, 0.0)
            scr6 = scrpool.tile([P, D], F32, tag="scrA")
            ACT(scr6[:], rlt[:], AF.Square, accum_out=norms2[:, NT + t:NT + t + 1])


        def s2_sheets(cc):
            sl = slice(TPC * cc, TPC * cc + TPC)
            TT = nc.vector.tensor_tensor
            TS = nc.vector.tensor_scalar
            STT = nc.vector.scalar_tensor_tensor

            # paired norm chain: norm = exp(.5 ln n2), 1/norm = exp(-.5 ln n2)
            nc.scalar.activation(pair2(lnn2, cc), pair2(norms2, cc), AF.Ln)
            nc.scalar.activation(pair2(rnorm2, cc), pair2(lnn2, cc), AF.Exp, scale=-0.5)
            nc.scalar.activation(pair2(zpair, cc), pair2(lnn2, cc), AF.Exp, scale=0.5)
            sqcb = sqc_s[:, sl].unsqueeze(1).broadcast_to((P, 2, TPC))
            rsqcb = rsqc_s[:, sl].unsqueeze(1).broadcast_to((P, 2, TPC))
            TT(pair2(zpair, cc), pair2(zpair, cc), sqcb, ALU.mult)
            # tanh via exp (stay in lnexp set): E = exp(2z); t = (E-1)/(E+1)
            nc.scalar.activation(pair2(lnn2, cc), pair2(zpair, cc), AF.Exp, scale=2.0)
            TS(pair2(lnn2, cc), pair2(lnn2, cc), 3.0e37, None, ALU.min)
            TS(pair2(zpair, cc), pair2(lnn2, cc), 1.0, None, ALU.add)
            nc.vector.reciprocal_approx_fast(pair2(norms2, cc),
                                                 pair2(zpair, cc))
            nc.vector._custom_dve(HYP_TFE, out=pair2(tpair, cc),
                                  in0=pair2(lnn2, cc), in1=pair2(norms2, cc))
            # f = tanh(z)/z = tanh(z) * (1/norm) * (1/sqc)
            TT(pair2(fpair, cc), pair2(tpair, cc), pair2(rnorm2, cc), ALU.mult)
            TT(pair2(fpair, cc), pair2(fpair, cc), rsqcb, ALU.mult)
            # s = tanh(z)/sqc ; q = s^2  (x2 | y2)
            TT(pair2(spair, cc), pair2(tpair, cc), rsqcb, ALU.mult)
            TT(pair2(qpair, cc), pair2(spair, cc), pair2(spair, cc), ALU.mult)

            q2 = qpair[:].rearrange("p (two t) -> p two t", two=2)
            x2 = q2[:, 0, TPC * cc:TPC * cc + TPC]
            y2 = q2[:, 1, TPC * cc:TPC * cc + TPC]
            f2 = fpair[:].rearrange("p (two t) -> p two t", two=2)
            fa = f2[:, 0, TPC * cc:TPC * cc + TPC]
            fr = f2[:, 1, TPC * cc:TPC * cc + TPC]

            xy = scr_s1
            TT(xy[:, sl], fa, fr, ALU.mult)
            TT(xy[:, sl], xy[:, sl], xy0[:, sl], ALU.mult)

            cxy2, cy2, cx2 = scr_s2, scr_s3, scr_s4
            STT(cxy2[:, sl], xy[:, sl], 2.0, c_s[:, sl], ALU.mult, ALU.mult)
            TT(cy2[:, sl], c_s[:, sl], y2, ALU.mult)
            TT(cx2[:, sl], c_s[:, sl], x2, ALU.mult)
            ccx2y2 = scr_s5
            TT(ccx2y2[:, sl], cx2[:, sl], cy2[:, sl], ALU.mult)
            n1, dm = scr_s6, scr_s3  # cy2 consumed after n1
            nc.vector._custom_dve(HYP_ADD1M, out=n1[:, sl], in0=cxy2[:, sl],
                                  in1=cy2[:, sl], imm2=-3.0e38)
            nc.vector._custom_dve(HYP_ADD1M, out=dm[:, sl], in0=cxy2[:, sl],
                                  in1=ccx2y2[:, sl], imm2=MINN)
            n2c = scr_s2  # cxy2 dead
            TS(n2c[:, sl], cx2[:, sl], -1.0, 1.0, ALU.mult, ALU.add)
            rdm = scr_s5  # ccx2y2 dead
            nc.vector.reciprocal_approx_fast(rdm[:, sl], dm[:, sl])
            a0, b0 = scr_s3, scr_s4  # dm, cx2 dead
            TT(a0[:, sl], n1[:, sl], rdm[:, sl], ALU.mult)
            TT(b0[:, sl], n2c[:, sl], rdm[:, sl], ALU.mult)

            # rn2 = a0^2*x2 + b0^2*y2 + 2*a0*b0*xy
            p1, p2 = scr_s6, scr_s2  # n1, n2c dead
            nc.vector._custom_dve(HYP_SQMUL, out=p1[:, sl], in0=a0[:, sl], in1=x2)
            nc.vector._custom_dve(HYP_SQMUL, out=p2[:, sl], in0=b0[:, sl], in1=y2)
            rn2 = scr_s5  # rdm dead
            TT(rn2[:, sl], p1[:, sl], p2[:, sl], ALU.add)
            ab = scr_s6  # p1 dead
            TT(ab[:, sl], a0[:, sl], b0[:, sl], ALU.mult)
            TT(ab[:, sl], ab[:, sl], xy[:, sl], ALU.mult)
            STT(rn2[:, sl], ab[:, sl], 2.0, rn2[:, sl], ALU.mult, ALU.add)
            # g = min(maxn/rn, 1) with 1/rn = exp(-0.5 ln rn2)
            lnr = scr_s2
            nc.scalar.activation(lnr[:, sl], rn2[:, sl], AF.Ln)
            rrn = scr_s6
            nc.scalar.activation(rrn[:, sl], lnr[:, sl], AF.Exp, scale=-0.5)
            g = scr_s2
            STT(g[:, sl], rrn[:, sl], BALL, rsqc_s[:, sl], ALU.mult, ALU.mult)
            TS(g[:, sl], g[:, sl], 1.0, None, ALU.min)

            TT(alpha_s[:, sl], g[:, sl], a0[:, sl], ALU.mult)
            TT(alpha_s[:, sl], alpha_s[:, sl], fa, ALU.mult)
            TT(beta_s[:, sl], g[:, sl], b0[:, sl], ALU.mult)
            TT(beta_s[:, sl], beta_s[:, sl], fr, ALU.mult)
            gg = scr_s6  # rrn dead
            nc.vector._custom_dve(HYP_SQMUL, out=gg[:, sl], in0=g[:, sl],
                                  in1=rn2[:, sl])
            TT(A_s[:, sl], gg[:, sl], c_s[:, sl], ALU.mult)
            TS(c2_s[:, sl], A_s[:, sl], -1.0, 1.0, ALU.mult, ALU.add)

        def a3_sweep(t, cc, xT):
            q = t - TPC * cc
            rlt3 = inpool.tile([P, D], F32, tag="rel")
            nc.sync.dma_start(rlt3[:], rel_in[t * 128:(t + 1) * 128, :])
            res = apool.tile([P, D], F32, tag="res")
            nc.vector._custom_dve(HYP_WSUM, out=res[:], in0=att_tiles[t][:],
                                  in1=rlt3[:],
                                  s0=alpha_s[:, t:t + 1], s1=beta_s[:, t:t + 1])
            if debug:
                nc.sync.dma_start(res_out[t * 128:(t + 1) * 128, :], res[:])
            ptp = pp_tp.tile([P, D], F32, tag="tp")
            for dk in range(4):
                nc.tensor.transpose(ptp[:, dk * 128:(dk + 1) * 128],
                                    res[:, dk * 128:(dk + 1) * 128], ident[:])
            nc.scalar.copy(
                xT[:].rearrange("p (dk n) -> p dk n", dk=4)[:, :, q * 128:(q + 1) * 128],
                ptp[:].rearrange("p (dk n) -> p dk n", dk=4))
            del att_tiles[t]

        def b_prep(cc):
            # ---- prep: tail norms + raw transposes (streamed)
            vhatT = xpool.tile([P, 4 * NS], F32, tag="vhatT")
            for q in range(TPC):
                r0 = cc * CS + q * 128
                vt = inpool.tile([P, D], F32, tag="vtl")
                nc.sync.dma_start(vt[:], tail_in[r0:r0 + 128, :])
                scr = scrpool.tile([P, D], F32, tag="scrA")
                ACT(scr[:], vt[:], AF.Square,
                    accum_out=vn2_sh[:, TPC * cc + q:TPC * cc + q + 1])
                ptp = pp_tp.tile([P, D], F32, tag="tp")
                for dk in range(4):
                    nc.tensor.transpose(ptp[:, dk * 128:(dk + 1) * 128],
                                        vt[:, dk * 128:(dk + 1) * 128], ident[:])
                nc.scalar.copy(
                    vhatT[:].rearrange("p (dk n) -> p dk n", dk=4)[:, :, q * 128:(q + 1) * 128],
                    ptp[:].rearrange("p (dk n) -> p dk n", dk=4))
            sl = slice(TPC * cc, TPC * cc + TPC)
            # vn = exp(0.5 ln vn2)  (stay in lnexp set)
            nc.scalar.activation(scr_s1[:, sl], vn2_sh[:, sl], AF.Ln)
            nc.scalar.activation(vn_sh[:, sl], scr_s1[:, sl], AF.Exp, scale=0.5)
            # vn sheet slice -> DRAM scratch -> partition-broadcast load
            scr_rows = vn_scr[cc * CS:(cc + 1) * CS, :]
            nc.sync.dma_start(scr_rows.rearrange("(t p) o -> p (t o)", p=P),
                              vn_sh[:, sl])
            vn_b = bpool.tile([P, NS], F32, tag="vn_b")
            nc.sync.dma_start(
                vn_b[:],
                scr_rows.rearrange("(o n) one -> o (n one)", o=1
                                   ).broadcast_to((P, NS)))
            rv_b = bpool1.tile([P, NS], F32, tag="rv_b")
            nc.vector.reciprocal_approx_fast(rv_b[:], vn_b[:])
            # normalize vhatT in place (one big strided TT)
            nc.vector.tensor_tensor(
                vhatT[:].rearrange("p (dk n) -> p dk n", dk=4),
                vhatT[:].rearrange("p (dk n) -> p dk n", dk=4),
                rv_b[:].unsqueeze(1).broadcast_to((P, 4, NS)), ALU.mult)

            tb_b = bpool1.tile([P, NS], F32, tag="tb_b")
            nc.sync.dma_start(
                tb_b[:],
                tb_in[cc * CS:(cc + 1) * CS, :].rearrange("(o n) one -> o (n one)", o=1
                                                          ).broadcast_to((P, NS)))
            return vhatT, vn_b, tb_b

        def b_main(cc, xT, vhatT, vn_b, tb_b, interleave=None):
            # ---- main sweep: function-major groups to batch ACT table sets
            NB = 4
            for g in range(TPC // NB):
                qs = list(range(g * NB, (g + 1) * NB))
                u_tiles = {}
                s_tiles = {}
                # pass 1: matmul, tanh, den/rden/u (ACT: Tanh only)
                for q in qs:
                    t = TPC * cc + q
                    tcol = slice(t, t + 1)
                    pmm = pp_mm.tile([P, NS], F32, tag="mm")
                    for ns in range(2):
                        for dk in range(4):
                            nc.tensor.matmul(
                                pmm[:, ns * 512:(ns + 1) * 512],
                                xT[:, dk * 1024 + q * 128: dk * 1024 + (q + 1) * 128],
                                vhatT[:, dk * 1024 + ns * 512: dk * 1024 + (ns + 1) * 512],
                                start=(dk == 0), stop=(dk == 3))
                    tt = bpool.tile([P, NS], F32, tag="bt")
                    nc.scalar.activation(tt[:], vn_b[:], AF.Tanh, scale=sqc_s[:, tcol])
                    den = bchain.tile([P, NS], F32, tag="bw1")
                    nc.vector._custom_dve(HYP_DEN, out=den[:], in0=pmm[:], in1=tt[:],
                                          s0=m2sqc_s[:, tcol], s1=A_s[:, tcol])
                    rden = bchain.tile([P, NS], F32, tag="bw2")
                    nc.vector.reciprocal_approx_fast(rden[:], den[:])
                    u = bheld.tile([P, NS], F32, tag="bu")
                    nc.vector._custom_dve(HYP_U, out=u[:], in0=rden[:], in1=tt[:],
                                          s0=c2_s[:, tcol], imm2=UMIN)
                    u_tiles[q] = u
                # pass 2: s = sqrt(1-u)  (ACT: Sqrt only)
                for q in qs:
                    s_ = bheld.tile([P, NS], dt.bfloat16, tag="bs")
                    nc.scalar.activation(s_[:], u_tiles[q][:], AF.Sqrt,
                                         bias=1.0, scale=-1.0)
                    s_tiles[q] = s_
                # pass 3: ln, ln, score, +tb, store  (ACT: Ln only)
                for q in qs:
                    t = TPC * cc + q
                    tcol = slice(t, t + 1)
                    l1 = bchain.tile([P, NS], F32, tag="bw1")
                    nc.scalar.activation(l1[:], s_tiles[q][:], AF.Ln, bias=1.0)
                    l2 = bchain.tile([P, NS], F32, tag="bw2")
                    nc.scalar.activation(l2[:], u_tiles[q][:], AF.Ln)
                    sc0 = bchain.tile([P, NS], F32, tag="bw1")
                    nc.vector._custom_dve(HYP_SCORE, out=sc0[:], in0=l1[:], in1=l2[:],
                                          s0=m4c_s[:, tcol], s1=khb[:, tcol], imm2=0.5)
                    outt = bchain.tile([P, NS], F32, tag="bw2")
                    nc.gpsimd.tensor_tensor(outt[:], sc0[:], tb_b[:], ALU.add)
                    nc.sync.dma_start(score_out[t * 128:(t + 1) * 128, :], outt[:])
                if interleave is not None:
                    interleave(g)

        # ---------------- emission: software-pipelined A phases (ACT one
        # tile ahead of DVE consumers), then B phases back to back.
        pre_state = {}
        for t in range(NT):
            a_pre(t)
            if t > 0:
                a_main(t - 1)
            if t == NT - 1:
                a_main(t)
            if t == TPC - 1:
                s2_after = True
        s2_sheets(0)
        s2_sheets(1)
        for cc in range(CPC):
            prep = b_prep(cc)
            xT = xtpool.tile([P, 4 * NS], F32, tag="xT", name="xT")
            for t in range(TPC * cc, TPC * cc + TPC):
                a3_sweep(t, cc, xT)
            b_main(cc, xT, *prep)

        if debug:
            dbg = [c_s, sqc_s, A_s, c2_s, alpha_s, beta_s]
            for i, sh in enumerate(dbg):
                nc.sync.dma_start(sh_out[:, i * NT:(i + 1) * NT], sh[:])

    nc.finalize()
    return nc


_NC_CACHE = {}


def _get_nc(debug=False):
    if debug not in _NC_CACHE:
        _NC_CACHE[debug] = build_nc(debug)
    return _NC_CACHE[debug]


def kernel(head, head_bias, rel, rel_diag, curvature, context, scale, tail,
           tail_bias, chunk_size, neg_sample_size, _debug=False, _trace=False):
    cs = int(chunk_size)
    ns = int(neg_sample_size)
    assert cs == CS and ns == NS, (cs, ns)
    head = np.ascontiguousarray(np.asarray(head, np.float32))
    head_bias = np.ascontiguousarray(np.asarray(head_bias, np.float32))
    rel = np.ascontiguousarray(np.asarray(rel, np.float32))
    rel_diag = np.ascontiguousarray(np.asarray(rel_diag, np.float32))
    curvature = np.ascontiguousarray(np.asarray(curvature, np.float32))
    context = np.ascontiguousarray(np.asarray(context, np.float32))
    scale = np.ascontiguousarray(np.asarray(scale, np.float32)).reshape(1, 1)
    tail = np.ascontiguousarray(np.asarray(tail, np.float32))
    tail_bias = np.ascontiguousarray(np.asarray(tail_bias, np.float32))

    nc = _get_nc(_debug)
    in_maps = []
    for core in range(NCORES):
        r = slice(core * BC, (core + 1) * BC)
        in_maps.append({
            "head": head[r], "head_bias": head_bias[r], "rel": rel[r],
            "rel_diag": rel_diag[r], "curvature": curvature[r],
            "context": context[r], "scale": scale, "tail": tail[r],
            "tail_bias": tail_bias[r],
        })
    res = run_bass_kernel_spmd(nc, in_maps, core_ids=list(range(NCORES)),
                               trace=_trace)
    score = np.concatenate([res.results[c]["score"] for c in range(NCORES)], axis=0)
    out = score.reshape(NCHUNK, CS, NS)
    if _debug:
        dbg_res = np.concatenate([res.results[c]["dbg_res"] for c in range(NCORES)], 0)
        dbg_sheets = [res.results[c]["dbg_sheets"] for c in range(NCORES)]
        return out, dbg_res, dbg_sheets
    if _trace:
        return out, res
    return out


# revision 49
# speedup vs baseline: 279.8897x; 1.0037x over previous
"""Trainium2 Bass kernel for nn_ATTHScore (hyperbolic attention KNN scoring).

Self-contained: shards the full inputs across 8 NeuronCores (2 chunks of
1024 rows per core - pure data parallel), runs a Bass/Tile kernel per core,
gathers the full (16, 1024, 1024) score tensor.

Math notes (algebraically identical to the reference, numerically closer to
f64 truth):
  With t = tanh(sqrt(c)*||v||), xv = <x, v/||v||>, A = c*||x||^2, c2 = 1-A:
    den = 1 - 2*sqrt(c)*t*xv + A*t^2
    u   = clip(c2*(1-t^2)/den, umin, 1)        # = 1 - artanh_arg^2
    artanh(arg) = ln(1+sqrt(1-u)) - ln(u)/2
    score = -(4/c)*artanh^2 + head_bias + tail_bias
  using the gyro-identity  num^2 = denom*(c1-c2)  which collapses the
  reference's cancellation-prone num/denom expression.

ACT table-set discipline (switches cost ~2.7us): 1/sqrt(x) is computed as
Exp(-0.5*Ln(x)) so phase A lives in natural_log_exp_and_others; Sigmoid and
all Tanh live in sigmoid_and_others; only the B-phase big Sqrt(1-u) uses
sqrt_and_others.
"""

import numpy as np

import concourse.bacc as bacc
import concourse.mybir as mybir
import concourse.tile as tile
import concourse.dve_ops as dve_ops
from concourse import masks
from concourse.dve_spec import Spec, Src0, Src1, C0, C1, C2, One, sq, maxx, minn, lower
from concourse.dve_uop import DveOpSpec
from concourse.bass_utils import run_bass_kernel_spmd
from contextlib import ExitStack

dt = mybir.dt
AF = mybir.ActivationFunctionType
ALU = mybir.AluOpType

# ---------------------------------------------------------------- constants
NCORES = 8
D = 512            # feature dim
CS = 1024          # chunk_size
NS = 1024          # neg_sample_size
NCHUNK = 16        # total chunks
CPC = NCHUNK // NCORES   # chunks per core = 2
BC = CPC * CS      # rows per core = 2048
NT = BC // 128     # row tiles per core = 16
TPC = CS // 128    # row tiles per chunk = 8
F32 = dt.float32

BALL = float(np.float32(1.0 - 1e-5))
UMIN = float(np.float32(1.0 - np.float64(np.float32(1.0 - 1e-5)) ** 2))
MINN = 1e-15

# ------------------------------------------------------- custom DVE ops


def _register_op(name: str, spec: Spec) -> "dve_ops.DveOp":
    for existing in dve_ops.OPS:
        if existing.name == name:
            return existing
    shas = {}
    for ver in ("v3", "v4"):
        uops = lower(spec, ver=ver)
        shas[ver] = DveOpSpec(name=name, opcode=0, uops=uops, rd1_en=True).sha(ver)
    op = dve_ops.DveOp(name, spec, subdim=False, uops_sha=shas)
    dve_ops.OPS.append(op)
    dve_ops.CUSTOM_DVE_SPECS[name] = spec
    dve_ops._SUB_OPCODE_FOR_NAME[name] = max(dve_ops._SUB_OPCODE_FOR_NAME.values()) + 1
    assert dve_ops._SUB_OPCODE_FOR_NAME[name] < 0x20
    return op


# den = 1 + (mm*t)*C0 + t^2*C1     (C0 = -2*sqrt(c), C1 = A)
HYP_DEN = _register_op("HYP_DEN", Spec(
    body=(Src0 * Src1) * C0 + sq(Src1) * C1 + One,
    reference=lambda in0, in1, s0, s1, imm2: in0 * in1 * s0 + in1 * in1 * s1 + 1.0,
))
# u = min(max((1 - t^2)*C0*rden, C2), 1)     (C0 = c2, C2 = umin)
HYP_U = _register_op("HYP_U", Spec(
    body=minn(maxx(((One - sq(Src1)) * C0) * Src0, C2), One),
    reference=lambda in0, in1, s0, s1, imm2: np.minimum(
        np.maximum((1.0 - in1 * in1) * s0 * in0, imm2), 1.0),
))
# score-tb = (l1 - l2*C2)^2*C0 + C1          (C0 = -4/c, C1 = hb, C2 = 0.5)
HYP_SCORE = _register_op("HYP_SCORE", Spec(
    body=sq(Src0 - Src1 * C2) * C0 + C1,
    reference=lambda in0, in1, s0, s1, imm2: (in0 - in1 * imm2) ** 2 * s0 + s1,
))
# out = Src0*C0 + Src1*C1  (per-partition weighted sum)
HYP_WSUM = _register_op("HYP_WSUM", Spec(
    body=Src0 * C0 + Src1 * C1,
    reference=lambda in0, in1, s0, s1, imm2: in0 * s0 + in1 * s1,
))
# out = sq(Src0) + sq(Src1)  (pair norm^2)
HYP_N2 = _register_op("HYP_N2", Spec(
    body=sq(Src0) + sq(Src1),
    reference=lambda in0, in1, s0, s1, imm2: in0 * in0 + in1 * in1,
))
# tanh from exp: t = (E - 1) * rE1   (E pre-clamped; rE1 = 1/(E+1))
HYP_TFE = _register_op("HYP_TFE", Spec(
    body=(Src0 - One) * Src1,
    reference=lambda in0, in1, s0, s1, imm2: (in0 - 1.0) * in1,
))
# out = max(Src0 + Src1 + 1, C2)   (n1 with C2=-inf-ish, dm with C2=MINN)
HYP_ADD1M = _register_op("HYP_ADD1M", Spec(
    body=maxx(Src0 + Src1 + One, C2),
    reference=lambda in0, in1, s0, s1, imm2: np.maximum(in0 + in1 + 1.0, imm2),
))
# out = sq(Src0) * Src1   (a0^2*x2, b0^2*y2, g^2*rn2)
HYP_SQMUL = _register_op("HYP_SQMUL", Spec(
    body=sq(Src0) * Src1,
    reference=lambda in0, in1, s0, s1, imm2: in0 * in0 * in1,
))


# ---------------------------------------------------------------- kernel IR


def build_nc(debug: bool = False):
    nc = bacc.Bacc("TRN2", target_bir_lowering=False, debug=False,
                   num_devices=NCORES)
    P = 128

    head_in = nc.declare_dram_parameter("head", [BC, D], F32, isOutput=False)
    hb_in = nc.declare_dram_parameter("head_bias", [BC, 1], F32, isOutput=False)
    rel_in = nc.declare_dram_parameter("rel", [BC, D], F32, isOutput=False)
    rd_in = nc.declare_dram_parameter("rel_diag", [BC, 2 * D], F32, isOutput=False)
    curv_in = nc.declare_dram_parameter("curvature", [BC, 1], F32, isOutput=False)
    ctx_in = nc.declare_dram_parameter("context", [BC, D], F32, isOutput=False)
    scale_in = nc.declare_dram_parameter("scale", [1, 1], F32, isOutput=False)
    tail_in = nc.declare_dram_parameter("tail", [BC, D], F32, isOutput=False)
    tb_in = nc.declare_dram_parameter("tail_bias", [BC, 1], F32, isOutput=False)
    score_out = nc.declare_dram_parameter("score", [BC, NS], F32, isOutput=True)
    vn_scr = nc.dram_tensor("vn_scratch", [BC, 1], F32)
    if debug:
        res_out = nc.declare_dram_parameter("dbg_res", [BC, D], F32, isOutput=True)
        sh_out = nc.declare_dram_parameter("dbg_sheets", [128, 6 * NT], F32, isOutput=True)

    def pairv(ap):
        return ap.rearrange("p (k two) -> p k two", two=2)

    with tile.TileContext(nc) as tc, ExitStack() as ctx:
        cpool = ctx.enter_context(tc.tile_pool(name="const", bufs=1))
        spool = ctx.enter_context(tc.tile_pool(name="sheets", bufs=1))
        apool = ctx.enter_context(tc.tile_pool(name="aflow", bufs=2))
        inpool = ctx.enter_context(tc.tile_pool(name="influx", bufs=2))
        hold = ctx.enter_context(tc.tile_pool(name="hold", bufs=TPC))
        wpool = ctx.enter_context(tc.tile_pool(name="wcols", bufs=3))
        bpool = ctx.enter_context(tc.tile_pool(name="bflow", bufs=2))
        bpool1 = ctx.enter_context(tc.tile_pool(name="bflow1", bufs=1))
        bchain = ctx.enter_context(tc.tile_pool(name="bchain", bufs=3))
        bheld = ctx.enter_context(tc.tile_pool(name="bheld", bufs=5))
        scrpool = ctx.enter_context(tc.tile_pool(name="scrp", bufs=1))
        xpool = ctx.enter_context(tc.tile_pool(name="xmats", bufs=1))
        xtpool = ctx.enter_context(tc.tile_pool(name="xtmat", bufs=2))
        pp_tp = ctx.enter_context(tc.tile_pool(name="ps_tp", bufs=2, space="PSUM"))
        pp_mm = ctx.enter_context(tc.tile_pool(name="ps_mm", bufs=3, space="PSUM"))

        ident = cpool.tile([P, P], F32)
        masks.make_identity(nc, ident[:])

        # ---- sheets: per-row scalars, col t = row-tile t, partition = row%128
        def sheet(nm, cols=NT):
            return spool.tile([P, cols], F32, tag=nm, name=nm)

        names = ("kcurv khb srot sref xy0 c_s sqc_s rsqc_s rc_s "
                 "m2sqc_s m4c_s A_s c2_s alpha_s beta_s wrot_s wref_s "
                 "scr_s1 scr_s2 scr_s3 scr_s4 scr_s5 scr_s6 vn2_sh vn_sh").split()
        (kcurv, khb, srot, sref, xy0, c_s, sqc_s, rsqc_s, rc_s,
         m2sqc_s, m4c_s, A_s, c2_s, alpha_s, beta_s, wrot_s, wref_s,
         scr_s1, scr_s2, scr_s3, scr_s4, scr_s5, scr_s6, vn2_sh, vn_sh) = [
            sheet(nm) for nm in names]
        # paired sheets (128, 2*NT): [att-side | rel-side]
        norms2 = sheet("norms2", 2 * NT)   # [na2 | nr2]
        lnn2 = sheet("lnn2", 2 * NT)
        rnorm2 = sheet("rnorm2", 2 * NT)   # 1/norm
        zpair = sheet("zpair", 2 * NT)     # z = sqc*norm
        tpair = sheet("tpair", 2 * NT)     # tanh(z)
        fpair = sheet("fpair", 2 * NT)     # tanh(z)/z
        spair = sheet("spair", 2 * NT)     # tanh(z)/sqc
        qpair = sheet("qpair", 2 * NT)     # (tanh(z)/sqc)^2 = x2|y2

        def pair2(sh, cc):
            # (128, 2, TPC) strided view of a (128, 2*NT) paired sheet
            return sh[:].rearrange("p (two t) -> p two t", two=2)[
                :, :, TPC * cc:TPC * cc + TPC]

        nc.sync.dma_start(kcurv[:], curv_in[:].rearrange("(t p) o -> p (t o)", p=P))
        nc.sync.dma_start(khb[:], hb_in[:].rearrange("(t p) o -> p (t o)", p=P))
        scale_bc = cpool.tile([P, 1], F32)
        nc.sync.dma_start(scale_bc[:], scale_in[:].broadcast_to((P, 1)))
        nscale_bc = cpool.tile([P, 1], F32)
        nc.vector.tensor_scalar(nscale_bc[:], scale_bc[:], -1.0, None, ALU.mult)

        ACT_LOAD(LNEXP_SET)
        # curvature chain (all 16 cols at once; natural_log_exp set)
        nc.scalar.activation(scr_s1[:], kcurv[:], AF.Exp)
        nc.scalar.activation(c_s[:], scr_s1[:], AF.Ln, bias=1.0)
        nc.scalar.activation(scr_s1[:], c_s[:], AF.Ln)
        nc.scalar.activation(sqc_s[:], scr_s1[:], AF.Exp, scale=0.5)
        nc.scalar.activation(rsqc_s[:], scr_s1[:], AF.Exp, scale=-0.5)
        nc.vector.tensor_tensor(rc_s[:], rsqc_s[:], rsqc_s[:], ALU.mult)
        nc.vector.tensor_scalar(m2sqc_s[:], sqc_s[:], -2.0, None, ALU.mult)
        nc.vector.tensor_scalar(m4c_s[:], rc_s[:], -4.0, None, ALU.mult)

        att_tiles = {}

        def a_sweep(t):
            """Load row-tile, build rot/ref (held), dots, rel-norm."""
            r0 = t * 128
            ht = inpool.tile([P, D], F32, tag="h")
            ct = inpool.tile([P, D], F32, tag="ctx")
            rdt = inpool.tile([P, 2 * D], F32, tag="rd")

            nc.sync.dma_start(ht[:], head_in[r0:r0 + 128, :])
            nc.sync.dma_start(ct[:], ctx_in[r0:r0 + 128, :])
            nc.sync.dma_start(rdt[:], rd_in[r0:r0 + 128, :])

            # pair-normalize rel_diag in place: gn = rd / sqrt(a^2+b^2)
            n2 = apool.tile([P, D], F32, tag="nA")
            nc.vector._custom_dve(HYP_N2, out=n2[:], in0=rdt[:, 0:2 * D:2],
                                  in1=rdt[:, 1:2 * D:2])
            lnv = apool.tile([P, D], F32, tag="nB")
            nc.scalar.activation(lnv[:], n2[:], AF.Ln)
            rsq = apool.tile([P, D], F32, tag="nA")
            nc.scalar.activation(rsq[:], lnv[:], AF.Exp, scale=-0.5)
            nc.gpsimd.tensor_tensor(
                pairv(rdt[:]), pairv(rdt[:]),
                rsq[:].unsqueeze(-1).broadcast_to((P, D, 2)), ALU.mult)

            # products (rot half on DVE, ref half on GPSIMD)
            hsw = pairv(ht[:])[:, :, ::-1]
            pr = apool.tile([P, D], F32, tag="rot")
            qr = apool.tile([P, D], F32, tag="qr")
            pf = apool.tile([P, D], F32, tag="ref")
            qf = apool.tile([P, D], F32, tag="qf")
            nc.vector.tensor_tensor(pr[:], rdt[:, 0:D], ht[:], ALU.mult)
            nc.vector.tensor_tensor(pairv(qr[:]), pairv(rdt[:, 0:D]), hsw, ALU.mult)
            nc.gpsimd.tensor_tensor(pf[:], rdt[:, D:2 * D], ht[:], ALU.mult)
            nc.gpsimd.tensor_tensor(pairv(qf[:]), pairv(rdt[:, D:2 * D]), hsw, ALU.mult)

            # combines (in place): pr becomes rot, pf becomes ref
            nc.vector.tensor_tensor(pr[:, 0:D:2], pr[:, 0:D:2], pr[:, 1:D:2], ALU.subtract)
            nc.vector.tensor_tensor(pr[:, 1:D:2], qr[:, 0:D:2], qr[:, 1:D:2], ALU.add)
            nc.gpsimd.tensor_tensor(pf[:, 0:D:2], pf[:, 0:D:2], pf[:, 1:D:2], ALU.add)
            nc.gpsimd.tensor_tensor(pf[:, 1:D:2], qf[:, 1:D:2], qf[:, 0:D:2], ALU.subtract)

            # attention dots (fused multiply-reduce on DVE)
            scr = scrpool.tile([P, D], F32, tag="scr")
            nc.vector.affine_mul_reduce(scr[:], srot[:, t:t + 1], ct[:], pr[:], 1.0, 0.0)
            scr2 = scrpool.tile([P, D], F32, tag="scr")
            nc.vector.affine_mul_reduce(scr2[:], sref[:, t:t + 1], ct[:], pf[:], 1.0, 0.0)

            # per-tile attention weights (exp-form sigmoid, lnexp set)
            wd = wpool.tile([P, 1], F32, tag="wd")
            nc.vector.tensor_tensor(wd[:], srot[:, t:t + 1], sref[:, t:t + 1],
                                    ALU.subtract)
            we = wpool.tile([P, 1], F32, tag="we")
            ACT(we[:], wd[:], AF.Exp, scale=nscale_bc[:])
            nc.vector.tensor_scalar(we[:], we[:], 1.0, None, ALU.add)
            wrot = wpool.tile([P, 1], F32, tag="wrot")
            nc.vector.reciprocal_approx_fast(wrot[:], we[:])
            wref = wpool.tile([P, 1], F32, tag="wref")
            nc.vector.tensor_scalar(wref[:], wrot[:], -1.0, 1.0, ALU.mult, ALU.add)

            # att + reductions (na2, xy0, nr2)
            rlt = inpool.tile([P, D], F32, tag="rel")
            nc.sync.dma_start(rlt[:], rel_in[r0:r0 + 128, :])
            att = hold.tile([P, D], F32, tag="att")
            att_tiles[t] = att
            nc.vector._custom_dve(HYP_WSUM, out=att[:], in0=pr[:], in1=pf[:],
                                  s0=wrot[:], s1=wref[:])
            scr4 = scrpool.tile([P, D], F32, tag="scrA")
            ACT(scr4[:], att[:], AF.Square, accum_out=norms2[:, t:t + 1])
            scr5 = scrpool.tile([P, D], F32, tag="scr")
            nc.vector.affine_mul_reduce(scr5[:], xy0[:, t:t + 1], att[:], rlt[:],
                                        1.0, 0.0)
            scr6 = scrpool.tile([P, D], F32, tag="scrA")
            ACT(scr6[:], rlt[:], AF.Square, accum_out=norms2[:, NT + t:NT + t + 1])


        def s2_sheets(cc):
            sl = slice(TPC * cc, TPC * cc + TPC)
            TT = nc.vector.tensor_tensor
            TS = nc.vector.tensor_scalar
            STT = nc.vector.scalar_tensor_tensor

            # paired norm chain: norm = exp(.5 ln n2), 1/norm = exp(-.5 ln n2)
            nc.scalar.activation(pair2(lnn2, cc), pair2(norms2, cc), AF.Ln)
            nc.scalar.activation(pair2(rnorm2, cc), pair2(lnn2, cc), AF.Exp, scale=-0.5)
            nc.scalar.activation(pair2(zpair, cc), pair2(lnn2, cc), AF.Exp, scale=0.5)
            sqcb = sqc_s[:, sl].unsqueeze(1).broadcast_to((P, 2, TPC))
            rsqcb = rsqc_s[:, sl].unsqueeze(1).broadcast_to((P, 2, TPC))
            TT(pair2(zpair, cc), pair2(zpair, cc), sqcb, ALU.mult)
            # tanh via exp (stay in lnexp set): E = exp(2z); t = (E-1)/(E+1)
            nc.scalar.activation(pair2(lnn2, cc), pair2(zpair, cc), AF.Exp, scale=2.0)
            TS(pair2(lnn2, cc), pair2(lnn2, cc), 3.0e37, None, ALU.min)
            TS(pair2(zpair, cc), pair2(lnn2, cc), 1.0, None, ALU.add)
            nc.vector.reciprocal_approx_fast(pair2(norms2, cc),
                                                 pair2(zpair, cc))
            nc.vector._custom_dve(HYP_TFE, out=pair2(tpair, cc),
                                  in0=pair2(lnn2, cc), in1=pair2(norms2, cc))
            # f = tanh(z)/z = tanh(z) * (1/norm) * (1/sqc)
            TT(pair2(fpair, cc), pair2(tpair, cc), pair2(rnorm2, cc), ALU.mult)
            TT(pair2(fpair, cc), pair2(fpair, cc), rsqcb, ALU.mult)
            # s = tanh(z)/sqc ; q = s^2  (x2 | y2)
            TT(pair2(spair, cc), pair2(tpair, cc), rsqcb, ALU.mult)
            TT(pair2(qpair, cc), pair2(spair, cc), pair2(spair, cc), ALU.mult)

            q2 = qpair[:].rearrange("p (two t) -> p two t", two=2)
            x2 = q2[:, 0, TPC * cc:TPC * cc + TPC]
            y2 = q2[:, 1, TPC * cc:TPC * cc + TPC]
            f2 = fpair[:].rearrange("p (two t) -> p two t", two=2)
            fa = f2[:, 0, TPC * cc:TPC * cc + TPC]
            fr = f2[:, 1, TPC * cc:TPC * cc + TPC]

            xy = scr_s1
            TT(xy[:, sl], fa, fr, ALU.mult)
            TT(xy[:, sl], xy[:, sl], xy0[:, sl], ALU.mult)

            cxy2, cy2, cx2 = scr_s2, scr_s3, scr_s4
            STT(cxy2[:, sl], xy[:, sl], 2.0, c_s[:, sl], ALU.mult, ALU.mult)
            TT(cy2[:, sl], c_s[:, sl], y2, ALU.mult)
            TT(cx2[:, sl], c_s[:, sl], x2, ALU.mult)
            ccx2y2 = scr_s5
            TT(ccx2y2[:, sl], cx2[:, sl], cy2[:, sl], ALU.mult)
            n1, dm = scr_s6, scr_s3  # cy2 consumed after n1
            nc.vector._custom_dve(HYP_ADD1M, out=n1[:, sl], in0=cxy2[:, sl],
                                  in1=cy2[:, sl], imm2=-3.0e38)
            nc.vector._custom_dve(HYP_ADD1M, out=dm[:, sl], in0=cxy2[:, sl],
                                  in1=ccx2y2[:, sl], imm2=MINN)
            n2c = scr_s2  # cxy2 dead
            TS(n2c[:, sl], cx2[:, sl], -1.0, 1.0, ALU.mult, ALU.add)
            rdm = scr_s5  # ccx2y2 dead
            nc.vector.reciprocal_approx_fast(rdm[:, sl], dm[:, sl])
            a0, b0 = scr_s3, scr_s4  # dm, cx2 dead
            TT(a0[:, sl], n1[:, sl], rdm[:, sl], ALU.mult)
            TT(b0[:, sl], n2c[:, sl], rdm[:, sl], ALU.mult)

            # rn2 = a0^2*x2 + b0^2*y2 + 2*a0*b0*xy
            p1, p2 = scr_s6, scr_s2  # n1, n2c dead
            nc.vector._custom_dve(HYP_SQMUL, out=p1[:, sl], in0=a0[:, sl], in1=x2)
            nc.vector._custom_dve(HYP_SQMUL, out=p2[:, sl], in0=b0[:, sl], in1=y2)
            rn2 = scr_s5  # rdm dead
            TT(rn2[:, sl], p1[:, sl], p2[:, sl], ALU.add)
            ab = scr_s6  # p1 dead
            TT(ab[:, sl], a0[:, sl], b0[:, sl], ALU.mult)
            TT(ab[:, sl], ab[:, sl], xy[:, sl], ALU.mult)
            STT(rn2[:, sl], ab[:, sl], 2.0, rn2[:, sl], ALU.mult, ALU.add)
            # g = min(maxn/rn, 1) with 1/rn = exp(-0.5 ln rn2)
            lnr = scr_s2
            nc.scalar.activation(lnr[:, sl], rn2[:, sl], AF.Ln)
            rrn = scr_s6
            nc.scalar.activation(rrn[:, sl], lnr[:, sl], AF.Exp, scale=-0.5)
            g = scr_s2
            STT(g[:, sl], rrn[:, sl], BALL, rsqc_s[:, sl], ALU.mult, ALU.mult)
            TS(g[:, sl], g[:, sl], 1.0, None, ALU.min)

            TT(alpha_s[:, sl], g[:, sl], a0[:, sl], ALU.mult)
            TT(alpha_s[:, sl], alpha_s[:, sl], fa, ALU.mult)
            TT(beta_s[:, sl], g[:, sl], b0[:, sl], ALU.mult)
            TT(beta_s[:, sl], beta_s[:, sl], fr, ALU.mult)
            gg = scr_s6  # rrn dead
            nc.vector._custom_dve(HYP_SQMUL, out=gg[:, sl], in0=g[:, sl],
                                  in1=rn2[:, sl])
            TT(A_s[:, sl], gg[:, sl], c_s[:, sl], ALU.mult)
            TS(c2_s[:, sl], A_s[:, sl], -1.0, 1.0, ALU.mult, ALU.add)

        def a3_sweep(t, cc, xT):
            q = t - TPC * cc
            rlt3 = inpool.tile([P, D], F32, tag="rel")
            nc.sync.dma_start(rlt3[:], rel_in[t * 128:(t + 1) * 128, :])
            res = apool.tile([P, D], F32, tag="res")
            nc.vector._custom_dve(HYP_WSUM, out=res[:], in0=att_tiles[t][:],
                                  in1=rlt3[:],
                                  s0=alpha_s[:, t:t + 1], s1=beta_s[:, t:t + 1])
            if debug:
                nc.sync.dma_start(res_out[t * 128:(t + 1) * 128, :], res[:])
            ptp = pp_tp.tile([P, D], F32, tag="tp")
            for dk in range(4):
                nc.tensor.transpose(ptp[:, dk * 128:(dk + 1) * 128],
                                    res[:, dk * 128:(dk + 1) * 128], ident[:])
            nc.scalar.copy(
                xT[:].rearrange("p (dk n) -> p dk n", dk=4)[:, :, q * 128:(q + 1) * 128],
                ptp[:].rearrange("p (dk n) -> p dk n", dk=4))
            del att_tiles[t]

        def b_prep(cc):
            # ---- prep: tail norms + raw transposes (streamed)
            vhatT = xpool.tile([P, 4 * NS], F32, tag="vhatT")
            for q in range(TPC):
                r0 = cc * CS + q * 128
                vt = inpool.tile([P, D], F32, tag="vtl")
                nc.sync.dma_start(vt[:], tail_in[r0:r0 + 128, :])
                scr = scrpool.tile([P, D], F32, tag="scrA")
                ACT(scr[:], vt[:], AF.Square,
                    accum_out=vn2_sh[:, TPC * cc + q:TPC * cc + q + 1])
                ptp = pp_tp.tile([P, D], F32, tag="tp")
                for dk in range(4):
                    nc.tensor.transpose(ptp[:, dk * 128:(dk + 1) * 128],
                                        vt[:, dk * 128:(dk + 1) * 128], ident[:])
                nc.scalar.copy(
                    vhatT[:].rearrange("p (dk n) -> p dk n", dk=4)[:, :, q * 128:(q + 1) * 128],
                    ptp[:].rearrange("p (dk n) -> p dk n", dk=4))
            sl = slice(TPC * cc, TPC * cc + TPC)
            # vn = exp(0.5 ln vn2)  (stay in lnexp set)
            nc.scalar.activation(scr_s1[:, sl], vn2_sh[:, sl], AF.Ln)
            nc.scalar.activation(vn_sh[:, sl], scr_s1[:, sl], AF.Exp, scale=0.5)
            # vn sheet slice -> DRAM scratch -> partition-broadcast load
            scr_rows = vn_scr[cc * CS:(cc + 1) * CS, :]
            nc.sync.dma_start(scr_rows.rearrange("(t p) o -> p (t o)", p=P),
                              vn_sh[:, sl])
            vn_b = bpool.tile([P, NS], F32, tag="vn_b")
            nc.sync.dma_start(
                vn_b[:],
                scr_rows.rearrange("(o n) one -> o (n one)", o=1
                                   ).broadcast_to((P, NS)))
            rv_b = bpool1.tile([P, NS], F32, tag="rv_b")
            nc.vector.reciprocal_approx_fast(rv_b[:], vn_b[:])
            # normalize vhatT in place (one big strided TT)
            nc.vector.tensor_tensor(
                vhatT[:].rearrange("p (dk n) -> p dk n", dk=4),
                vhatT[:].rearrange("p (dk n) -> p dk n", dk=4),
                rv_b[:].unsqueeze(1).broadcast_to((P, 4, NS)), ALU.mult)

            tb_b = bpool1.tile([P, NS], F32, tag="tb_b")
            nc.sync.dma_start(
                tb_b[:],
                tb_in[cc * CS:(cc + 1) * CS, :].rearrange("(o n) one -> o (n one)", o=1
                                                          ).broadcast_to((P, NS)))
            return vhatT, vn_b, tb_b

        def b_main(cc, xT, vhatT, vn_b, tb_b, interleave=None):
            # ---- main sweep: function-major groups to batch ACT table sets
            NB = 4
            for g in range(TPC // NB):
                qs = list(range(g * NB, (g + 1) * NB))
                u_tiles = {}
                s_tiles = {}
                # pass 1: matmul, tanh, den/rden/u (ACT: Tanh only)
                for q in qs:
                    t = TPC * cc + q
                    tcol = slice(t, t + 1)
                    pmm = pp_mm.tile([P, NS], F32, tag="mm")
                    for ns in range(2):
                        for dk in range(4):
                            nc.tensor.matmul(
                                pmm[:, ns * 512:(ns + 1) * 512],
                                xT[:, dk * 1024 + q * 128: dk * 1024 + (q + 1) * 128],
                                vhatT[:, dk * 1024 + ns * 512: dk * 1024 + (ns + 1) * 512],
                                start=(dk == 0), stop=(dk == 3))
                    tt = bpool.tile([P, NS], F32, tag="bt")
                    nc.scalar.activation(tt[:], vn_b[:], AF.Tanh, scale=sqc_s[:, tcol])
                    den = bchain.tile([P, NS], F32, tag="bw1")
                    nc.vector._custom_dve(HYP_DEN, out=den[:], in0=pmm[:], in1=tt[:],
                                          s0=m2sqc_s[:, tcol], s1=A_s[:, tcol])
                    rden = bchain.tile([P, NS], F32, tag="bw2")
                    nc.vector.reciprocal_approx_fast(rden[:], den[:])
                    u = bheld.tile([P, NS], F32, tag="bu")
                    nc.vector._custom_dve(HYP_U, out=u[:], in0=rden[:], in1=tt[:],
                                          s0=c2_s[:, tcol], imm2=UMIN)
                    u_tiles[q] = u
                # pass 2: s = sqrt(1-u)  (ACT: Sqrt only)
                for q in qs:
                    s_ = bheld.tile([P, NS], dt.bfloat16, tag="bs")
                    nc.scalar.activation(s_[:], u_tiles[q][:], AF.Sqrt,
                                         bias=1.0, scale=-1.0)
                    s_tiles[q] = s_
                # pass 3: ln, ln, score, +tb, store  (ACT: Ln only)
                for q in qs:
                    t = TPC * cc + q
                    tcol = slice(t, t + 1)
                    l1 = bchain.tile([P, NS], F32, tag="bw1")
                    nc.scalar.activation(l1[:], s_tiles[q][:], AF.Ln, bias=1.0)
                    l2 = bchain.tile([P, NS], F32, tag="bw2")
                    nc.scalar.activation(l2[:], u_tiles[q][:], AF.Ln)
                    sc0 = bchain.tile([P, NS], F32, tag="bw1")
                    nc.vector._custom_dve(HYP_SCORE, out=sc0[:], in0=l1[:], in1=l2[:],
                                          s0=m4c_s[:, tcol], s1=khb[:, tcol], imm2=0.5)
                    outt = bchain.tile([P, NS], F32, tag="bw2")
                    nc.gpsimd.tensor_tensor(outt[:], sc0[:], tb_b[:], ALU.add)
                    nc.sync.dma_start(score_out[t * 128:(t + 1) * 128, :], outt[:])
                if interleave is not None:
                    interleave(g)

        # ---------------- emission: software-pipelined A phases (ACT one
        # tile ahead of DVE consumers), then B phases back to back.
        pre_state = {}
        for t in range(NT):
            a_pre(t)
            if t > 0:
                a_main(t - 1)
            if t == NT - 1:
                a_main(t)
            if t == TPC - 1:
                s2_after = True
        s2_sheets(0)
        s2_sheets(1)
        for cc in range(CPC):
            prep = b_prep(cc)
            xT = xtpool.tile([P, 4 * NS], F32, tag="xT", name="xT")
            for t in range(TPC * cc, TPC * cc + TPC):
                a3_sweep(t, cc, xT)
            b_main(cc, xT, *prep)

        if debug:
            dbg = [c_s, sqc_s, A_s, c2_s, alpha_s, beta_s]
            for i, sh in enumerate(dbg):
                nc.sync.dma_start(sh_out[:, i * NT:(i + 1) * NT], sh[:])

    nc.finalize()
    return nc


_NC_CACHE = {}


def _get_nc(debug=False):
    if debug not in _NC_CACHE:
        _NC_CACHE[debug] = build_nc(debug)
    return _NC_CACHE[debug]


def kernel(head, head_bias, rel, rel_diag, curvature, context, scale, tail,
           tail_bias, chunk_size, neg_sample_size, _debug=False, _trace=False):
    cs = int(chunk_size)
    ns = int(neg_sample_size)
    assert cs == CS and ns == NS, (cs, ns)
    head = np.ascontiguousarray(np.asarray(head, np.float32))
    head_bias = np.ascontiguousarray(np.asarray(head_bias, np.float32))
    rel = np.ascontiguousarray(np.asarray(rel, np.float32))
    rel_diag = np.ascontiguousarray(np.asarray(rel_diag, np.float32))
    curvature = np.ascontiguousarray(np.asarray(curvature, np.float32))
    context = np.ascontiguousarray(np.asarray(context, np.float32))
    scale = np.ascontiguousarray(np.asarray(scale, np.float32)).reshape(1, 1)
    tail = np.ascontiguousarray(np.asarray(tail, np.float32))
    tail_bias = np.ascontiguousarray(np.asarray(tail_bias, np.float32))

    nc = _get_nc(_debug)
    in_maps = []
    for core in range(NCORES):
        r = slice(core * BC, (core + 1) * BC)
        in_maps.append({
            "head": head[r], "head_bias": head_bias[r], "rel": rel[r],
            "rel_diag": rel_diag[r], "curvature": curvature[r],
            "context": context[r], "scale": scale, "tail": tail[r],
            "tail_bias": tail_bias[r],
        })
    res = run_bass_kernel_spmd(nc, in_maps, core_ids=list(range(NCORES)),
                               trace=_trace)
    score = np.concatenate([res.results[c]["score"] for c in range(NCORES)], axis=0)
    out = score.reshape(NCHUNK, CS, NS)
    if _debug:
        dbg_res = np.concatenate([res.results[c]["dbg_res"] for c in range(NCORES)], 0)
        dbg_sheets = [res.results[c]["dbg_sheets"] for c in range(NCORES)]
        return out, dbg_res, dbg_sheets
    if _trace:
        return out, res
    return out
